# revision 25
# baseline (speedup 1.0000x reference)
"""Multi-head causal attention (B=4, S=2048, D=1024, H=16) on 8 TRN2 cores.

Sharding: tensor-parallel over heads (2 heads/core), proj_out row-parallel
with the cross-core reduction done host-side during unsharding.

Per-core kernel layout (all contractions on the SBUF partition axis):
  xT      (1024 d, 8192 tok)   host-pretransposed activations (shared input)
  qT/kT   (128 e2, 2048 s)     per batch; e2 = 2 heads x 64
  scoresT (128 sk, 512 sq)     kv-major scores -> exp -> PV matmul directly
  denom   ones-matmul broadcast of the per-column sums of exp(scores)
  ctxT    (128 e2, 512 sq)     normalized, fed straight into row-parallel Wo
  outp    (1024 o, 8192 tok)   per-core partial; host sums over cores

Head pairs run concurrently on the PE via automatic tile_position (row
tiles for the 64-contraction score matmuls, col tiles for the 64-wide
den/PV matmuls).  Pipeline: warmup matmuls keep the HAM clock-gate warm
through the initial DMA; qkv projection + deferred output-projection
work is interleaved at fine grain between attention iterations so the
PE never idles long enough to re-throttle.

v2: den/PV lag scores by 2 iterations (exp+mask off the PE critical
path), a filler precedes each score pair (LDWEIGHTS cover), output
DMAs move 2-chunk 2KB rows split across sync+gpsimd queues, warmup
deps issue first on the vector queue.
"""

import sys

if "/opt/trn_rl_repo" not in sys.path:
    sys.path.insert(0, "/opt/trn_rl_repo")

from contextlib import ExitStack

import numpy as np

import concourse.bass as bass
import concourse.bacc as bacc
import concourse.mybir as mybir
import concourse.tile as tile
from concourse.bass_utils import run_bass_kernel_spmd
from concourse.masks import make_identity

B, S, D, H, E = 4, 2048, 1024, 16, 64
NCORES = 8
HL = H // NCORES          # heads per core = 2
EL = HL * E               # local feature width = 128
SQ = 512                  # query chunk (matmul moving dim)
NQ = S // SQ              # 4
KT = 128                  # kv tile (contraction tile)
DT = 128                  # d-model contraction tile
ND = D // DT              # 8
N_WARM = 8                # HAM warmup matmuls during initial DMA
F32 = mybir.dt.float32
BF16 = mybir.dt.bfloat16
EXP = mybir.ActivationFunctionType.Exp

LAST_RESULTS = None


def build():
    nc = bacc.Bacc()
    xT = nc.declare_dram_parameter("xT", [B * NQ, DT, ND, SQ], BF16, isOutput=False)
    wqkv = nc.declare_dram_parameter("wqkv", [DT, ND, 3 * EL], BF16, isOutput=False)
    wo = nc.declare_dram_parameter("wo", [EL, D], BF16, isOutput=False)
    masks = nc.declare_dram_parameter("masks", [KT, NQ, 2, SQ], BF16, isOutput=False)
    outp = nc.declare_dram_parameter("outp", [D, B * S], BF16, isOutput=True)

    with tile.TileContext(nc) as tc, ExitStack() as ctx:
        consts = ctx.enter_context(tc.tile_pool(name="consts", bufs=1))
        xt_pool = ctx.enter_context(tc.tile_pool(name="xt", bufs=3))
        qk_pool = ctx.enter_context(tc.tile_pool(name="qk", bufs=2))
        ex_pool = ctx.enter_context(tc.tile_pool(name="ex", bufs=8))
        misc_pool = ctx.enter_context(tc.tile_pool(name="misc", bufs=2))
        out_pool = ctx.enter_context(tc.tile_pool(name="outsb", bufs=16))
        mm_psum = ctx.enter_context(tc.tile_pool(name="mmps", bufs=2, space="PSUM"))
        sc_psum = ctx.enter_context(tc.tile_pool(name="scps", bufs=2, space="PSUM"))
        acc_psum = ctx.enter_context(tc.tile_pool(name="accps", bufs=1, space="PSUM"))
        den_psum = ctx.enter_context(tc.tile_pool(name="denps", bufs=1, space="PSUM"))

        # DMA queue plan for the startup window: the sync (HWDGE) queue
        # carries the first half of wqkv, then the first x chunk's low
        # half, then the rest of wqkv -- so the t<4 projection matmuls
        # (and the wqkv-fed warmup) can start as early as possible.
        # masks/wo are only needed a few us in so they load later.
        wqkv_sb = consts.tile([DT, ND, 3 * EL], BF16)
        nc.sync.dma_start(wqkv_sb[:, 0:ND // 2, :], wqkv[:, 0:ND // 2, :])
        masks_sb = consts.tile([KT, NQ, 2, SQ], BF16)
        wo_sb = consts.tile([EL, D], BF16)
        ones_sb = consts.tile([DT, DT], BF16)
        nc.vector.memset(ones_sb[:], 1.0)
        ident = consts.tile([DT, DT], BF16)
        make_identity(nc, ident[:])

        qkv_tiles = {}
        xt_tiles = {}

        def load_chunk(b, c):
            xt8 = xt_pool.tile([DT, ND, SQ], BF16, name=f"xt_{b}_{c}", tag="xt")
            # x is host-prechunked to [chunk, p, t, n] so each load is one
            # contiguous 8KB-per-partition transfer; split across HWDGE
            # (sync) and SWDGE (gpsimd) so the two halves stream in
            # parallel and the t<4 matmuls start after half has landed
            half = ND // 2
            u = b * NQ + c
            nc.sync.dma_start(xt8[:, 0:half, :], xT[u, :, 0:half, :])
            nc.gpsimd.dma_start(xt8[:, half:ND, :], xT[u, :, half:ND, :])
            xt_tiles[(b, c)] = xt8
            return xt8

        load_chunk(0, 0)
        nc.sync.dma_start(wqkv_sb[:, ND // 2:ND, :], wqkv[:, ND // 2:ND, :])

        # HAM warmup: junk matmuls over the just-landed first wqkv half
        # keep the PE busy (no DVE dependency at all) through the rest of
        # the initial DMA window so the clock-gate is at 8/8 when the
        # first real matmul issues.
        warm_ps = mm_psum.tile([DT, SQ], F32, name="warm_ps", tag="mm")
        for _ in range(N_WARM):
            nc.tensor.matmul(
                warm_ps[:],
                wqkv_sb[:, 0, 0:DT],
                wqkv_sb[:, 0:2, 0:SQ // 2],
                start=True,
                stop=True,
            )

        def qkv_chunk_pieces(b, c):
            # returns fine-grained filler closures; each emits a small piece
            # of the qkv work for chunk (b, c) so it can be sprinkled between
            # attention iterations (engine FIFOs are strict in-order)
            if c == 0:
                qT = qk_pool.tile([EL, S], BF16, name=f"qT_{b}", tag="qT")
                kT = qk_pool.tile([EL, S], BF16, name=f"kT_{b}", tag="kT")
                vT = qk_pool.tile([EL, S], BF16, name=f"vT_{b}", tag="vT", bufs=1)
                v_sb = qk_pool.tile([KT, S // KT, EL], BF16, name=f"v_{b}", tag="v")
                qkv_tiles[b] = (qT, kT, vT, v_sb)
            qT, kT, vT, v_sb = qkv_tiles[b]
            xt8 = xt_tiles.get((b, c))
            if xt8 is None:
                xt8 = load_chunk(b, c)

            psums = {}

            def proj_piece(dest, col0, t0, t1):
                # emitted as col-tile pairs (two concurrent [128,64] tiles)
                # so fillers share the den/PV tile config -- avoids the
                # ~110ns PE array-reconfigure penalty per switch
                def go():
                    if t0 == 0:
                        psums[col0] = mm_psum.tile(
                            [EL, SQ], F32, name=f"qkv_ps_{b}_{c}_{col0}", tag="mm"
                        )
                    ps = psums[col0]
                    for t in range(t0, t1):
                        for h in range(2):
                            nc.tensor.matmul(
                                ps[h * E:(h + 1) * E, :],
                                wqkv_sb[:, t, col0 + h * E:col0 + (h + 1) * E],
                                xt8[:, t, :],
                                start=(t == 0),
                                stop=(t == ND - 1),
                                skip_group_check=True,
                            )
                    if t1 == ND:
                        nc.vector.tensor_copy(dest[:, c * SQ:(c + 1) * SQ], ps[:])
                return go

            def vtr4():
                # all four transposes in one piece: transpose-mode is its
                # own PE tile config, so batching them pays the reconfigure
                # penalty once per chunk instead of per pair
                def go():
                    for j in range(4 * c, 4 * c + 4):
                        vt_ps = mm_psum.tile([KT, KT], BF16, name=f"vt_ps_{b}_{j}", tag="mm")
                        nc.tensor.transpose(vt_ps[:], vT[:, j * KT:(j + 1) * KT], ident[:])
                        nc.vector.tensor_copy(v_sb[:, j, :], vt_ps[:])
                return go

            pieces = []
            for col0, dest in ((0, qT), (EL, kT), (2 * EL, vT)):
                for t0 in range(0, ND, 4):
                    pieces.append(proj_piece(dest, col0, t0, t0 + 4))
            pieces.append(vtr4())
            return pieces

        # global filler queue: (chunk_tag_or_None, closure).  Chunk pieces
        # and deferred output projections pop between attention iterations
        # at a self-balancing cadence; `reserve` pieces are held back so
        # later units never starve.
        fill_q = []
        # output staging: adjacent q-chunk pairs share one [DT, 2*SQ] tile
        # so each outp DMA moves 2KB rows (half the descriptor count);
        # pairs alternate between the sync and gpsimd queues.
        opairs = {}

        def pop_one():
            if fill_q:
                fill_q.pop(0)[1]()

        def emit_attn_unit(b, c, reserve=0, last=False):
            # returns tail closures (the row-parallel output projection) to
            # be deferred into later units' iteration loops
            qT, kT, vT, v_sb = qkv_tiles[b]
            J = (c + 1) * (SQ // KT)  # causal kv tiles for this chunk
            ctx_ps = acc_psum.tile([2 * E, SQ], F32, name=f"ctx_{b}_{c}", tag="ctx")
            denb = den_psum.tile([KT, SQ], F32, name=f"den_{b}_{c}", tag="den")
            def emit_denpv(idx, j, ex, cut):
                # denominator rides PE: ones.T @ ex accumulates the
                # per-column sums, already broadcast over partitions.
                # start/stop key on EMISSION order (idx), not kv index.
                for h in range(HL):
                    nc.tensor.matmul(
                        denb[h * E:(h + 1) * E, cut:SQ],
                        ones_sb[:, h * E:(h + 1) * E],
                        ex[:, h, cut:SQ],
                        start=(idx == 0),
                        stop=(idx == J - 1),
                        skip_group_check=True,
                    )
                for h in range(HL):
                    nc.tensor.matmul(
                        ctx_ps[h * E:(h + 1) * E, cut:SQ],
                        v_sb[:, j, h * E:(h + 1) * E],
                        ex[:, h, cut:SQ],
                        start=(idx == 0),
                        stop=(idx == J - 1),
                        skip_group_check=True,
                    )

            # diagonal kv tiles (small, exp/mask-paced) run FIRST while the
            # filler queue is full; the dense full-width tiles close the
            # unit back-to-back once fillers thin out
            js = list(range(max(0, J - 4), J)) + list(range(0, max(0, J - 4)))
            # kv tiles are processed in MACRO pairs with den/PV lagging by
            # one macro: the PE sees [sc,sc][den,pv,den,pv][fillers] with
            # only two tile-config switches (row<->col) per macro, the
            # second block of each kind paying no reconfigure penalty, and
            # exp (scalar) + mask (vector) a full macro off the critical
            # path.
            pending = []

            def emit_sc(idx, j):
                rdiag = j - (c * (SQ // KT))
                # columns [0, cut) of this q-chunk are fully masked for
                # diagonal kv tiles -- skip them everywhere
                cut = KT * rdiag if rdiag > 0 else 0
                n = SQ - cut
                sc = sc_psum.tile([KT, 2, SQ], F32, name=f"sc_{b}_{c}_{j}", tag="sc")
                ex = ex_pool.tile([KT, 2, SQ], BF16, name=f"ex_{b}_{c}_{j}", tag="ex")
                for h in range(HL):
                    nc.tensor.matmul(
                        sc[:, h, 0:n],
                        kT[h * E:(h + 1) * E, j * KT:(j + 1) * KT],
                        qT[h * E:(h + 1) * E, c * SQ + cut:(c + 1) * SQ],
                        start=True,
                        stop=True,
                    )
                nc.scalar.activation(
                    ex[:, :, cut:SQ], sc[:, :, 0:n], EXP, scale=0.125
                )
                if rdiag >= 0:
                    # mask rides the (mostly idle) gpsimd engine: on the
                    # vector queue it would sit behind filler casts and
                    # stall the den/PV matmuls at unit boundaries
                    nc.gpsimd.tensor_mul(
                        ex[:, :, cut:SQ],
                        ex[:, :, cut:SQ],
                        masks_sb[:, rdiag, :, cut:SQ],
                    )
                pending.append((idx, j, ex, cut))

            JM = J // 2
            for mi in range(JM):
                pop_one()
                emit_sc(2 * mi, js[2 * mi])
                emit_sc(2 * mi + 1, js[2 * mi + 1])
                while len(pending) > 2:
                    emit_denpv(*pending.pop(0))
                # self-balancing filler cadence: spread the queue (minus
                # the held-back reserve) evenly over remaining macros
                quota = -(-max(0, len(fill_q) - reserve) // (JM - mi)) - 1
                for _ in range(quota):
                    pop_one()
            for p in pending:
                emit_denpv(*p)

            recb = misc_pool.tile([KT, SQ], F32, name=f"rec_{b}_{c}", tag="recb")
            nc.vector.reciprocal_approx_fast(recb[:], denb[:])
            ctx_sb = misc_pool.tile(
                [2 * E, SQ], BF16, name=f"ctxsb_{b}_{c}", tag="ctxsb", bufs=3
            )
            nc.vector.tensor_mul(ctx_sb[:], ctx_ps[:], recb[:])

            # ---- row-parallel output projection (partial), deferred ----
            # These pieces pop during later units.  When they land in an
            # ACT-idle phase (after a c==3 unit, or the final flush), the
            # PSUM->SBUF copy goes to the scalar engine so the vector
            # engine's copy backlog doesn't stall the mm_psum rotation;
            # the final flush also borrows the (now idle) score banks.
            def oproj_piece(o, scalar_copy=False, use_sc_psum=False):
                def go():
                    pool = sc_psum if use_sc_psum else mm_psum
                    tag = "sc" if use_sc_psum else "mm"
                    ops = pool.tile([DT, SQ], F32, name=f"op_{b}_{c}_{o}", tag=tag)
                    for h in range(2):
                        nc.tensor.matmul(
                            ops[h * E:(h + 1) * E, :],
                            wo_sb[:, o * DT + h * E:o * DT + (h + 1) * E],
                            ctx_sb[:],
                            start=True, stop=True,
                            skip_group_check=True,
                        )
                    key = (b, c // 2, o)
                    ent = opairs.get(key)
                    if ent is None:
                        ent = opairs[key] = [
                            out_pool.tile(
                                [DT, 2, SQ], BF16, name=f"osb_{b}_{c // 2}_{o}",
                                tag="osb",
                            ),
                            0,
                        ]
                    osb = ent[0]
                    if scalar_copy:
                        nc.scalar.activation(
                            osb[:, c % 2, :], ops[:], mybir.ActivationFunctionType.Copy
                        )
                    else:
                        nc.vector.tensor_copy(osb[:, c % 2, :], ops[:])
                    ent[1] += 1
                    if ent[1] == 2:
                        del opairs[key]
                        if last:
                            # kernel tail: exp work is done, so the scalar
                            # queue is free to help drain the final outputs
                            q = (nc.sync, nc.gpsimd, nc.scalar)[o % 3]
                        else:
                            q = nc.sync if o % 2 == 0 else nc.gpsimd
                        q.dma_start(
                            outp[
                                o * DT:(o + 1) * DT,
                                b * S + (c // 2) * 2 * SQ: b * S + (c // 2 + 1) * 2 * SQ,
                            ],
                            osb[:],
                        )
                return go

            return [
                oproj_piece(
                    o,
                    scalar_copy=(last and o % 2 == 1),
                    use_sc_psum=(last and o % 2 == 1),
                )
                for o in range(D // DT)
            ]

        # software pipeline: the global queue runs two qkv chunks ahead of
        # the attention units, plus deferred output projections.  The last
        # batch's units are rotated so the final unit is a small one (J=4)
        # and the kernel tail stays dense.
        NU = B * NQ
        unit_order = [(b, c) for b in range(B) for c in range(NQ)]
        unit_order = unit_order[:-NQ] + unit_order[-NQ + 1:] + [unit_order[-NQ]]
        chunk_order = [(b, c) for b in range(B) for c in range(NQ)]

        for p in qkv_chunk_pieces(0, 0):
            p()
        nc.gpsimd.dma_start(masks_sb[:], masks[:])
        nc.sync.dma_start(wo_sb[:], wo[:])
        fill_q += [((0, 1), p) for p in qkv_chunk_pieces(0, 1)]

        for i, (b, c) in enumerate(unit_order):
            if i + 2 < NU:
                ch = chunk_order[i + 2]
                fill_q += [(ch, p) for p in qkv_chunk_pieces(*ch)]
            # guard: every chunk this unit reads must be emitted before
            # the unit's first score matmul
            while any(
                t is not None and t[0] == b and t[1] <= c for t, _ in fill_q
            ):
                pop_one()
            tail = emit_attn_unit(
                b, c, reserve=8 if i < NU - 2 else 0, last=(i == NU - 1)
            )
            fill_q += [(None, p) for p in tail]
        while fill_q:
            pop_one()

    nc.finalize()
    return nc


def _host_inputs(x, Wq, Wk, Wv, Wo):
    import ml_dtypes

    bf = ml_dtypes.bfloat16
    # [chunk, p, t, n]: per-chunk contiguous tiles of x^T
    xT = np.ascontiguousarray(
        x.reshape(B * NQ, SQ, ND, DT).transpose(0, 3, 2, 1)
    ).astype(bf)
    p = np.arange(KT)[:, None, None]
    rr = np.arange(NQ)[None, :, None]
    cc = np.arange(SQ)[None, None, :]
    masks = (cc >= KT * rr + p).astype(bf)
    # duplicated per head so the mask multiply is one [KT, 2, n] DVE op
    masks = np.ascontiguousarray(np.repeat(masks[:, :, None, :], 2, axis=2))
    in_maps = []
    for core in range(NCORES):
        hs = slice(core * HL, (core + 1) * HL)
        wq = Wq[hs].reshape(EL, D).T
        wk = Wk[hs].reshape(EL, D).T
        wv = Wv[hs].reshape(EL, D).T
        wqkv = np.ascontiguousarray(
            np.concatenate([wq, wk, wv], axis=1)
            .reshape(ND, DT, 3 * EL)
            .transpose(1, 0, 2)
        ).astype(bf)
        woL = np.ascontiguousarray(
            Wo[:, core * EL:(core + 1) * EL].T
        ).astype(bf)
        in_maps.append({"xT": xT, "wqkv": wqkv, "wo": woL, "masks": masks})
    return in_maps


def kernel(x, Wq, Wk, Wv, Wo):
    global LAST_RESULTS
    x, Wq, Wk, Wv, Wo = (np.asarray(a, dtype=np.float32) for a in (x, Wq, Wk, Wv, Wo))
    nc = build()
    in_maps = _host_inputs(x, Wq, Wk, Wv, Wo)
    import os
    res = run_bass_kernel_spmd(
        nc, in_maps, list(range(NCORES)),
        trace=bool(os.environ.get("BASS_KERNEL_TRACE")),
    )
    LAST_RESULTS = res
    acc = np.zeros((D, B * S), np.float32)
    for rmap in res.results:
        acc += rmap["outp"]
    return np.ascontiguousarray(acc.T).reshape(B, S, D)


if __name__ == "__main__":
    rng = np.random.default_rng(0)
    scale = 1.0 / np.sqrt(D)
    x = rng.standard_normal((B, S, D), dtype=np.float32)
    Wq = rng.standard_normal((H, E, D), dtype=np.float32) * scale
    Wk = rng.standard_normal((H, E, D), dtype=np.float32) * scale
    Wv = rng.standard_normal((H, E, D), dtype=np.float32) * scale
    Wo = rng.standard_normal((D, D), dtype=np.float32) * scale
    out = kernel(x, Wq, Wk, Wv, Wo)
    print(out.shape, out.dtype, float(np.abs(out).max()))



# revision 26
# speedup vs baseline: 1.2771x; 1.2771x over previous
"""Multi-head causal attention (B=4, S=2048, D=1024, H=16) on 8 TRN2 cores.

Sharding: tensor-parallel over heads (2 heads/core), proj_out row-parallel
with the cross-core reduction done host-side during unsharding.

Per-core kernel layout (all contractions on the SBUF partition axis):
  xT      (1024 d, 8192 tok)   host-pretransposed activations (shared input)
  qT/kT   (128 e2, 2048 s)     per batch; e2 = 2 heads x 64
  scoresT (128 sk, 512 sq)     kv-major scores -> exp -> PV matmul directly
  denom   ones-matmul broadcast of the per-column sums of exp(scores)
  ctxT    (128 e2, 512 sq)     normalized, fed straight into row-parallel Wo
  outp    (1024 o, 8192 tok)   per-core partial; host sums over cores

Head pairs run concurrently on the PE via automatic tile_position (row
tiles for the 64-contraction score matmuls, col tiles for the 64-wide
den/PV matmuls).  Pipeline: warmup matmuls keep the HAM clock-gate warm
through the initial DMA; qkv projection + deferred output-projection
work is interleaved at fine grain between attention iterations so the
PE never idles long enough to re-throttle.

v2: den/PV lag scores by 2 iterations (exp+mask off the PE critical
path), a filler precedes each score pair (LDWEIGHTS cover), output
DMAs move 2-chunk 2KB rows split across sync+gpsimd queues, warmup
deps issue first on the vector queue.
"""

import sys

if "/opt/trn_rl_repo" not in sys.path:
    sys.path.insert(0, "/opt/trn_rl_repo")

from contextlib import ExitStack

import numpy as np

import concourse.bass as bass
import concourse.bacc as bacc
import concourse.mybir as mybir
import concourse.tile as tile
from concourse.bass_utils import run_bass_kernel_spmd
from concourse.masks import make_identity

B, S, D, H, E = 4, 2048, 1024, 16, 64
NCORES = 8
HL = H // NCORES          # heads per core = 2
EL = HL * E               # local feature width = 128
SQ = 512                  # query chunk (matmul moving dim)
NQ = S // SQ              # 4
KT = 128                  # kv tile (contraction tile)
DT = 128                  # d-model contraction tile
ND = D // DT              # 8
N_WARM = 8                # HAM warmup matmuls during initial DMA
F32 = mybir.dt.float32
BF16 = mybir.dt.bfloat16
EXP = mybir.ActivationFunctionType.Exp

LAST_RESULTS = None


def build():
    nc = bacc.Bacc()
    xT = nc.declare_dram_parameter("xT", [B * NQ, DT, ND, SQ], BF16, isOutput=False)
    wqkv = nc.declare_dram_parameter("wqkv", [DT, ND, 3 * EL], BF16, isOutput=False)
    wo = nc.declare_dram_parameter("wo", [EL, D], BF16, isOutput=False)
    masks = nc.declare_dram_parameter("masks", [KT, NQ, 2, SQ], BF16, isOutput=False)
    outp = nc.declare_dram_parameter("outp", [D, B * S], BF16, isOutput=True)

    with tile.TileContext(nc) as tc, ExitStack() as ctx:
        consts = ctx.enter_context(tc.tile_pool(name="consts", bufs=1))
        xt_pool = ctx.enter_context(tc.tile_pool(name="xt", bufs=3))
        qk_pool = ctx.enter_context(tc.tile_pool(name="qk", bufs=2))
        ex_pool = ctx.enter_context(tc.tile_pool(name="ex", bufs=8))
        misc_pool = ctx.enter_context(tc.tile_pool(name="misc", bufs=2))
        out_pool = ctx.enter_context(tc.tile_pool(name="outsb", bufs=16))
        mm_psum = ctx.enter_context(tc.tile_pool(name="mmps", bufs=2, space="PSUM"))
        sc_psum = ctx.enter_context(tc.tile_pool(name="scps", bufs=2, space="PSUM"))
        acc_psum = ctx.enter_context(tc.tile_pool(name="accps", bufs=1, space="PSUM"))
        den_psum = ctx.enter_context(tc.tile_pool(name="denps", bufs=1, space="PSUM"))

        # DMA queue plan for the startup window: the sync (HWDGE) queue
        # carries the first half of wqkv, then the first x chunk's low
        # half, then the rest of wqkv -- so the t<4 projection matmuls
        # (and the wqkv-fed warmup) can start as early as possible.
        # masks/wo are only needed a few us in so they load later.
        wqkv_sb = consts.tile([DT, ND, 3 * EL], BF16)
        nc.sync.dma_start(wqkv_sb[:, 0:ND // 2, :], wqkv[:, 0:ND // 2, :])
        masks_sb = consts.tile([KT, NQ, 2, SQ], BF16)
        wo_sb = consts.tile([EL, D], BF16)
        ones_sb = consts.tile([DT, DT], BF16)
        nc.vector.memset(ones_sb[:], 1.0)
        ident = consts.tile([DT, DT], BF16)
        make_identity(nc, ident[:])

        qkv_tiles = {}
        xt_tiles = {}

        def load_chunk(b, c):
            xt8 = xt_pool.tile([DT, ND, SQ], BF16, name=f"xt_{b}_{c}", tag="xt")
            # x is host-prechunked to [chunk, p, t, n] so each load is one
            # contiguous 8KB-per-partition transfer; split across HWDGE
            # (sync) and SWDGE (gpsimd) so the two halves stream in
            # parallel and the t<4 matmuls start after half has landed
            half = ND // 2
            u = b * NQ + c
            nc.sync.dma_start(xt8[:, 0:half, :], xT[u, :, 0:half, :])
            nc.gpsimd.dma_start(xt8[:, half:ND, :], xT[u, :, half:ND, :])
            xt_tiles[(b, c)] = xt8
            return xt8

        load_chunk(0, 0)
        nc.sync.dma_start(wqkv_sb[:, ND // 2:ND, :], wqkv[:, ND // 2:ND, :])

        # HAM warmup: junk matmuls over the just-landed first wqkv half
        # keep the PE busy (no DVE dependency at all) through the rest of
        # the initial DMA window so the clock-gate is at 8/8 when the
        # first real matmul issues.
        warm_ps = mm_psum.tile([DT, SQ], F32, name="warm_ps", tag="mm")
        for _ in range(N_WARM):
            nc.tensor.matmul(
                warm_ps[:],
                wqkv_sb[:, 0, 0:DT],
                wqkv_sb[:, 0:2, 0:SQ // 2],
                start=True,
                stop=True,
            )

        def qkv_chunk_pieces(b, c):
            # returns fine-grained filler closures; each emits a small piece
            # of the qkv work for chunk (b, c) so it can be sprinkled between
            # attention iterations (engine FIFOs are strict in-order)
            if c == 0:
                qT = qk_pool.tile([EL, S], BF16, name=f"qT_{b}", tag="qT")
                kT = qk_pool.tile([EL, S], BF16, name=f"kT_{b}", tag="kT")
                vT = qk_pool.tile([EL, S], BF16, name=f"vT_{b}", tag="vT", bufs=1)
                v_sb = qk_pool.tile([KT, S // KT, EL], BF16, name=f"v_{b}", tag="v")
                qkv_tiles[b] = (qT, kT, vT, v_sb)
            qT, kT, vT, v_sb = qkv_tiles[b]
            xt8 = xt_tiles.get((b, c))
            if xt8 is None:
                xt8 = load_chunk(b, c)

            psums = {}

            def proj_piece(dest, col0, t0, t1):
                # emitted as col-tile pairs (two concurrent [128,64] tiles)
                # so fillers share the den/PV tile config -- avoids the
                # ~110ns PE array-reconfigure penalty per switch
                def go():
                    if t0 == 0:
                        psums[col0] = mm_psum.tile(
                            [EL, SQ], F32, name=f"qkv_ps_{b}_{c}_{col0}", tag="mm"
                        )
                    ps = psums[col0]
                    for t in range(t0, t1):
                        for h in range(2):
                            nc.tensor.matmul(
                                ps[h * E:(h + 1) * E, :],
                                wqkv_sb[:, t, col0 + h * E:col0 + (h + 1) * E],
                                xt8[:, t, :],
                                start=(t == 0),
                                stop=(t == ND - 1),
                                skip_group_check=True,
                            )
                    if t1 == ND:
                        nc.vector.tensor_copy(dest[:, c * SQ:(c + 1) * SQ], ps[:])
                return go

            def vtr4():
                # all four transposes in one piece: transpose-mode is its
                # own PE tile config, so batching them pays the reconfigure
                # penalty once per chunk instead of per pair
                def go():
                    for j in range(4 * c, 4 * c + 4):
                        vt_ps = mm_psum.tile([KT, KT], BF16, name=f"vt_ps_{b}_{j}", tag="mm")
                        nc.tensor.transpose(vt_ps[:], vT[:, j * KT:(j + 1) * KT], ident[:])
                        nc.vector.tensor_copy(v_sb[:, j, :], vt_ps[:])
                return go

            pieces = []
            for col0, dest in ((0, qT), (EL, kT), (2 * EL, vT)):
                for t0 in range(0, ND, 4):
                    pieces.append(proj_piece(dest, col0, t0, t0 + 4))
            pieces.append(vtr4())
            return pieces

        # global filler queue: (chunk_tag_or_None, closure).  Chunk pieces
        # and deferred output projections pop between attention iterations
        # at a self-balancing cadence; `reserve` pieces are held back so
        # later units never starve.
        fill_q = []
        # output staging: adjacent q-chunk pairs share one [DT, 2*SQ] tile
        # so each outp DMA moves 2KB rows (half the descriptor count);
        # pairs alternate between the sync and gpsimd queues.
        opairs = {}

        def pop_one():
            if fill_q:
                fill_q.pop(0)[1]()

        def emit_attn_unit(b, c, reserve=0, last=False):
            # returns tail closures (the row-parallel output projection) to
            # be deferred into later units' iteration loops
            qT, kT, vT, v_sb = qkv_tiles[b]
            J = (c + 1) * (SQ // KT)  # causal kv tiles for this chunk
            ctx_ps = acc_psum.tile([2 * E, SQ], F32, name=f"ctx_{b}_{c}", tag="ctx")
            denb = den_psum.tile([KT, SQ], F32, name=f"den_{b}_{c}", tag="den")
            def emit_denpv(idx, j, ex, cut):
                # denominator rides PE: ones.T @ ex accumulates the
                # per-column sums, already broadcast over partitions.
                # start/stop key on EMISSION order (idx), not kv index.
                for h in range(HL):
                    nc.tensor.matmul(
                        denb[h * E:(h + 1) * E, cut:SQ],
                        ones_sb[:, h * E:(h + 1) * E],
                        ex[:, h, cut:SQ],
                        start=(idx == 0),
                        stop=(idx == J - 1),
                        skip_group_check=True,
                    )
                for h in range(HL):
                    nc.tensor.matmul(
                        ctx_ps[h * E:(h + 1) * E, cut:SQ],
                        v_sb[:, j, h * E:(h + 1) * E],
                        ex[:, h, cut:SQ],
                        start=(idx == 0),
                        stop=(idx == J - 1),
                        skip_group_check=True,
                    )

            # diagonal kv tiles (small, exp/mask-paced) run FIRST while the
            # filler queue is full; the dense full-width tiles close the
            # unit back-to-back once fillers thin out
            js = list(range(max(0, J - 4), J)) + list(range(0, max(0, J - 4)))
            # kv tiles are processed in MACRO pairs with den/PV lagging by
            # one macro: the PE sees [sc,sc][den,pv,den,pv][fillers] with
            # only two tile-config switches (row<->col) per macro, the
            # second block of each kind paying no reconfigure penalty, and
            # exp (scalar) + mask (vector) a full macro off the critical
            # path.
            pending = []

            def emit_sc(idx, j):
                rdiag = j - (c * (SQ // KT))
                # columns [0, cut) of this q-chunk are fully masked for
                # diagonal kv tiles -- skip them everywhere
                cut = KT * rdiag if rdiag > 0 else 0
                n = SQ - cut
                sc = sc_psum.tile([KT, 2, SQ], F32, name=f"sc_{b}_{c}_{j}", tag="sc")
                ex = ex_pool.tile([KT, 2, SQ], BF16, name=f"ex_{b}_{c}_{j}", tag="ex")
                for h in range(HL):
                    nc.tensor.matmul(
                        sc[:, h, 0:n],
                        kT[h * E:(h + 1) * E, j * KT:(j + 1) * KT],
                        qT[h * E:(h + 1) * E, c * SQ + cut:(c + 1) * SQ],
                        start=True,
                        stop=True,
                    )
                nc.scalar.activation(
                    ex[:, :, cut:SQ], sc[:, :, 0:n], EXP, scale=0.125
                )
                if rdiag >= 0:
                    nc.vector.tensor_mul(
                        ex[:, :, cut:SQ],
                        ex[:, :, cut:SQ],
                        masks_sb[:, rdiag, :, cut:SQ],
                    )
                pending.append((idx, j, ex, cut))

            JM = J // 2
            for mi in range(JM):
                pop_one()
                emit_sc(2 * mi, js[2 * mi])
                emit_sc(2 * mi + 1, js[2 * mi + 1])
                while len(pending) > 2:
                    emit_denpv(*pending.pop(0))
                # self-balancing filler cadence: spread the queue (minus
                # the held-back reserve) evenly over remaining macros
                quota = -(-max(0, len(fill_q) - reserve) // (JM - mi)) - 1
                for _ in range(quota):
                    pop_one()
            for p in pending:
                emit_denpv(*p)

            recb = misc_pool.tile([KT, SQ], F32, name=f"rec_{b}_{c}", tag="recb")
            nc.vector.reciprocal_approx_fast(recb[:], denb[:])
            ctx_sb = misc_pool.tile(
                [2 * E, SQ], BF16, name=f"ctxsb_{b}_{c}", tag="ctxsb", bufs=3
            )
            nc.vector.tensor_mul(ctx_sb[:], ctx_ps[:], recb[:])

            # ---- row-parallel output projection (partial), deferred ----
            # These pieces pop during later units.  When they land in an
            # ACT-idle phase (after a c==3 unit, or the final flush), the
            # PSUM->SBUF copy goes to the scalar engine so the vector
            # engine's copy backlog doesn't stall the mm_psum rotation;
            # the final flush also borrows the (now idle) score banks.
            def oproj_piece(o, scalar_copy=False, use_sc_psum=False):
                def go():
                    pool = sc_psum if use_sc_psum else mm_psum
                    tag = "sc" if use_sc_psum else "mm"
                    ops = pool.tile([DT, SQ], F32, name=f"op_{b}_{c}_{o}", tag=tag)
                    for h in range(2):
                        nc.tensor.matmul(
                            ops[h * E:(h + 1) * E, :],
                            wo_sb[:, o * DT + h * E:o * DT + (h + 1) * E],
                            ctx_sb[:],
                            start=True, stop=True,
                            skip_group_check=True,
                        )
                    key = (b, c // 2, o)
                    ent = opairs.get(key)
                    if ent is None:
                        ent = opairs[key] = [
                            out_pool.tile(
                                [DT, 2, SQ], BF16, name=f"osb_{b}_{c // 2}_{o}",
                                tag="osb",
                            ),
                            0,
                        ]
                    osb = ent[0]
                    if scalar_copy:
                        nc.scalar.activation(
                            osb[:, c % 2, :], ops[:], mybir.ActivationFunctionType.Copy
                        )
                    else:
                        nc.vector.tensor_copy(osb[:, c % 2, :], ops[:])
                    ent[1] += 1
                    if ent[1] == 2:
                        del opairs[key]
                        if last:
                            # kernel tail: exp work is done, so the scalar
                            # queue is free to help drain the final outputs
                            q = (nc.sync, nc.gpsimd, nc.scalar)[o % 3]
                        else:
                            q = nc.sync if o % 2 == 0 else nc.gpsimd
                        q.dma_start(
                            outp[
                                o * DT:(o + 1) * DT,
                                b * S + (c // 2) * 2 * SQ: b * S + (c // 2 + 1) * 2 * SQ,
                            ],
                            osb[:],
                        )
                return go

            return [
                oproj_piece(
                    o,
                    scalar_copy=(last and o % 2 == 1),
                    use_sc_psum=(last and o % 2 == 1),
                )
                for o in range(D // DT)
            ]

        # software pipeline: the global queue runs two qkv chunks ahead of
        # the attention units, plus deferred output projections.  The last
        # batch's units are rotated so the final unit is a small one (J=4)
        # and the kernel tail stays dense.
        NU = B * NQ
        unit_order = [(b, c) for b in range(B) for c in range(NQ)]
        unit_order = unit_order[:-NQ] + unit_order[-NQ + 1:] + [unit_order[-NQ]]
        chunk_order = [(b, c) for b in range(B) for c in range(NQ)]

        for p in qkv_chunk_pieces(0, 0):
            p()
        nc.gpsimd.dma_start(masks_sb[:], masks[:])
        nc.sync.dma_start(wo_sb[:], wo[:])
        fill_q += [((0, 1), p) for p in qkv_chunk_pieces(0, 1)]

        for i, (b, c) in enumerate(unit_order):
            if i + 2 < NU:
                ch = chunk_order[i + 2]
                fill_q += [(ch, p) for p in qkv_chunk_pieces(*ch)]
            # guard: every chunk this unit reads must be emitted before
            # the unit's first score matmul
            while any(
                t is not None and t[0] == b and t[1] <= c for t, _ in fill_q
            ):
                pop_one()
            tail = emit_attn_unit(
                b, c, reserve=8 if i < NU - 2 else 0, last=(i == NU - 1)
            )
            fill_q += [(None, p) for p in tail]
        while fill_q:
            pop_one()

    nc.finalize()
    return nc


def _host_inputs(x, Wq, Wk, Wv, Wo):
    import ml_dtypes

    bf = ml_dtypes.bfloat16
    # [chunk, p, t, n]: per-chunk contiguous tiles of x^T
    xT = np.ascontiguousarray(
        x.reshape(B * NQ, SQ, ND, DT).transpose(0, 3, 2, 1)
    ).astype(bf)
    p = np.arange(KT)[:, None, None]
    rr = np.arange(NQ)[None, :, None]
    cc = np.arange(SQ)[None, None, :]
    masks = (cc >= KT * rr + p).astype(bf)
    # duplicated per head so the mask multiply is one [KT, 2, n] DVE op
    masks = np.ascontiguousarray(np.repeat(masks[:, :, None, :], 2, axis=2))
    in_maps = []
    for core in range(NCORES):
        hs = slice(core * HL, (core + 1) * HL)
        wq = Wq[hs].reshape(EL, D).T
        wk = Wk[hs].reshape(EL, D).T
        wv = Wv[hs].reshape(EL, D).T
        wqkv = np.ascontiguousarray(
            np.concatenate([wq, wk, wv], axis=1)
            .reshape(ND, DT, 3 * EL)
            .transpose(1, 0, 2)
        ).astype(bf)
        woL = np.ascontiguousarray(
            Wo[:, core * EL:(core + 1) * EL].T
        ).astype(bf)
        in_maps.append({"xT": xT, "wqkv": wqkv, "wo": woL, "masks": masks})
    return in_maps


def kernel(x, Wq, Wk, Wv, Wo):
    global LAST_RESULTS
    x, Wq, Wk, Wv, Wo = (np.asarray(a, dtype=np.float32) for a in (x, Wq, Wk, Wv, Wo))
    nc = build()
    in_maps = _host_inputs(x, Wq, Wk, Wv, Wo)
    import os
    res = run_bass_kernel_spmd(
        nc, in_maps, list(range(NCORES)),
        trace=bool(os.environ.get("BASS_KERNEL_TRACE")),
    )
    LAST_RESULTS = res
    acc = np.zeros((D, B * S), np.float32)
    for rmap in res.results:
        acc += rmap["outp"]
    return np.ascontiguousarray(acc.T).reshape(B, S, D)


if __name__ == "__main__":
    rng = np.random.default_rng(0)
    scale = 1.0 / np.sqrt(D)
    x = rng.standard_normal((B, S, D), dtype=np.float32)
    Wq = rng.standard_normal((H, E, D), dtype=np.float32) * scale
    Wk = rng.standard_normal((H, E, D), dtype=np.float32) * scale
    Wv = rng.standard_normal((H, E, D), dtype=np.float32) * scale
    Wo = rng.standard_normal((D, D), dtype=np.float32) * scale
    out = kernel(x, Wq, Wk, Wv, Wo)
    print(out.shape, out.dtype, float(np.abs(out).max()))



# revision 27
# speedup vs baseline: 1.2915x; 1.0112x over previous
"""Multi-head causal attention (B=4, S=2048, D=1024, H=16) on 8 TRN2 cores.

Sharding: tensor-parallel over heads (2 heads/core), proj_out row-parallel
with the cross-core reduction done host-side during unsharding.

Per-core kernel layout (all contractions on the SBUF partition axis):
  xT      (1024 d, 8192 tok)   host-pretransposed activations (shared input)
  qT/kT   (128 e2, 2048 s)     per batch; e2 = 2 heads x 64
  scoresT (128 sk, 512 sq)     kv-major scores -> exp -> PV matmul directly
  denom   ones-matmul broadcast of the per-column sums of exp(scores)
  ctxT    (128 e2, 512 sq)     normalized, fed straight into row-parallel Wo
  outp    (1024 o, 8192 tok)   per-core partial; host sums over cores

Head pairs run concurrently on the PE via automatic tile_position (row
tiles for the 64-contraction score matmuls, col tiles for the 64-wide
den/PV matmuls).  Pipeline: warmup matmuls keep the HAM clock-gate warm
through the initial DMA; qkv projection + deferred output-projection
work is interleaved at fine grain between attention iterations so the
PE never idles long enough to re-throttle.

v2: den/PV lag scores by 2 iterations (exp+mask off the PE critical
path), a filler precedes each score pair (LDWEIGHTS cover), output
DMAs move 2-chunk 2KB rows split across sync+gpsimd queues, warmup
deps issue first on the vector queue.
"""

import sys

if "/opt/trn_rl_repo" not in sys.path:
    sys.path.insert(0, "/opt/trn_rl_repo")

from contextlib import ExitStack

import numpy as np

import concourse.bass as bass
import concourse.bacc as bacc
import concourse.mybir as mybir
import concourse.tile as tile
from concourse.bass_utils import run_bass_kernel_spmd
from concourse.masks import make_identity

B, S, D, H, E = 4, 2048, 1024, 16, 64
NCORES = 8
HL = H // NCORES          # heads per core = 2
EL = HL * E               # local feature width = 128
SQ = 512                  # query chunk (matmul moving dim)
NQ = S // SQ              # 4
KT = 128                  # kv tile (contraction tile)
DT = 128                  # d-model contraction tile
ND = D // DT              # 8
N_WARM = 8                # HAM warmup matmuls during initial DMA
F32 = mybir.dt.float32
BF16 = mybir.dt.bfloat16
EXP = mybir.ActivationFunctionType.Exp

LAST_RESULTS = None


def build():
    nc = bacc.Bacc()
    xT = nc.declare_dram_parameter("xT", [B * NQ, DT, ND, SQ], BF16, isOutput=False)
    wqkv = nc.declare_dram_parameter("wqkv", [DT, ND, 3 * EL], BF16, isOutput=False)
    wo = nc.declare_dram_parameter("wo", [EL, D], BF16, isOutput=False)
    masks = nc.declare_dram_parameter("masks", [KT, NQ, 2, SQ], BF16, isOutput=False)
    outp = nc.declare_dram_parameter("outp", [D, B * S], BF16, isOutput=True)

    with tile.TileContext(nc) as tc, ExitStack() as ctx:
        consts = ctx.enter_context(tc.tile_pool(name="consts", bufs=1))
        xt_pool = ctx.enter_context(tc.tile_pool(name="xt", bufs=3))
        qk_pool = ctx.enter_context(tc.tile_pool(name="qk", bufs=2))
        ex_pool = ctx.enter_context(tc.tile_pool(name="ex", bufs=8))
        misc_pool = ctx.enter_context(tc.tile_pool(name="misc", bufs=2))
        out_pool = ctx.enter_context(tc.tile_pool(name="outsb", bufs=16))
        mm_psum = ctx.enter_context(tc.tile_pool(name="mmps", bufs=2, space="PSUM"))
        sc_psum = ctx.enter_context(tc.tile_pool(name="scps", bufs=2, space="PSUM"))
        acc_psum = ctx.enter_context(tc.tile_pool(name="accps", bufs=1, space="PSUM"))
        den_psum = ctx.enter_context(tc.tile_pool(name="denps", bufs=1, space="PSUM"))

        # DMA queue plan for the startup window: the sync (HWDGE) queue
        # carries the first half of wqkv, then the first x chunk's low
        # half, then the rest of wqkv -- so the t<4 projection matmuls
        # (and the wqkv-fed warmup) can start as early as possible.
        # masks/wo are only needed a few us in so they load later.
        wqkv_sb = consts.tile([DT, ND, 3 * EL], BF16)
        nc.sync.dma_start(wqkv_sb[:, 0:ND // 2, :], wqkv[:, 0:ND // 2, :])
        masks_sb = consts.tile([KT, NQ, 2, SQ], BF16)
        wo_sb = consts.tile([EL, D], BF16)
        ones_sb = consts.tile([DT, DT], BF16)
        nc.vector.memset(ones_sb[:], 1.0)
        ident = consts.tile([DT, DT], BF16)
        make_identity(nc, ident[:])

        qkv_tiles = {}
        xt_tiles = {}

        def load_chunk(b, c):
            xt8 = xt_pool.tile([DT, ND, SQ], BF16, name=f"xt_{b}_{c}", tag="xt")
            # x is host-prechunked to [chunk, p, t, n] so each load is one
            # contiguous 8KB-per-partition transfer; split across HWDGE
            # (sync) and SWDGE (gpsimd) so the two halves stream in
            # parallel and the t<4 matmuls start after half has landed
            half = ND // 2
            u = b * NQ + c
            nc.sync.dma_start(xt8[:, 0:half, :], xT[u, :, 0:half, :])
            nc.gpsimd.dma_start(xt8[:, half:ND, :], xT[u, :, half:ND, :])
            xt_tiles[(b, c)] = xt8
            return xt8

        load_chunk(0, 0)
        nc.sync.dma_start(wqkv_sb[:, ND // 2:ND, :], wqkv[:, ND // 2:ND, :])

        # HAM warmup: junk matmuls over the just-landed first wqkv half
        # keep the PE busy (no DVE dependency at all) through the rest of
        # the initial DMA window so the clock-gate is at 8/8 when the
        # first real matmul issues.
        warm_ps = mm_psum.tile([DT, SQ], F32, name="warm_ps", tag="mm")
        for _ in range(N_WARM):
            nc.tensor.matmul(
                warm_ps[:],
                wqkv_sb[:, 0, 0:DT],
                wqkv_sb[:, 0:2, 0:SQ // 2],
                start=True,
                stop=True,
            )

        def qkv_chunk_pieces(b, c):
            # returns fine-grained filler closures; each emits a small piece
            # of the qkv work for chunk (b, c) so it can be sprinkled between
            # attention iterations (engine FIFOs are strict in-order)
            if c == 0:
                qT = qk_pool.tile([EL, S], BF16, name=f"qT_{b}", tag="qT")
                kT = qk_pool.tile([EL, S], BF16, name=f"kT_{b}", tag="kT")
                vT = qk_pool.tile([EL, S], BF16, name=f"vT_{b}", tag="vT", bufs=1)
                v_sb = qk_pool.tile([KT, S // KT, EL], BF16, name=f"v_{b}", tag="v")
                qkv_tiles[b] = (qT, kT, vT, v_sb)
            qT, kT, vT, v_sb = qkv_tiles[b]
            xt8 = xt_tiles.get((b, c))
            if xt8 is None:
                xt8 = load_chunk(b, c)

            psums = {}

            def proj_piece(dest, col0, t0, t1):
                # emitted as col-tile pairs (two concurrent [128,64] tiles)
                # so fillers share the den/PV tile config -- avoids the
                # ~110ns PE array-reconfigure penalty per switch
                def go():
                    if t0 == 0:
                        psums[col0] = mm_psum.tile(
                            [EL, SQ], F32, name=f"qkv_ps_{b}_{c}_{col0}", tag="mm"
                        )
                    ps = psums[col0]
                    for t in range(t0, t1):
                        for h in range(2):
                            nc.tensor.matmul(
                                ps[h * E:(h + 1) * E, :],
                                wqkv_sb[:, t, col0 + h * E:col0 + (h + 1) * E],
                                xt8[:, t, :],
                                start=(t == 0),
                                stop=(t == ND - 1),
                                skip_group_check=True,
                            )
                    if t1 == ND:
                        nc.vector.tensor_copy(dest[:, c * SQ:(c + 1) * SQ], ps[:])
                return go

            def vtr4():
                # all four transposes in one piece: transpose-mode is its
                # own PE tile config, so batching them pays the reconfigure
                # penalty once per chunk instead of per pair
                def go():
                    for j in range(4 * c, 4 * c + 4):
                        vt_ps = mm_psum.tile([KT, KT], BF16, name=f"vt_ps_{b}_{j}", tag="mm")
                        nc.tensor.transpose(vt_ps[:], vT[:, j * KT:(j + 1) * KT], ident[:])
                        nc.vector.tensor_copy(v_sb[:, j, :], vt_ps[:])
                return go

            pieces = []
            for col0, dest in ((0, qT), (EL, kT), (2 * EL, vT)):
                for t0 in range(0, ND, 4):
                    pieces.append(proj_piece(dest, col0, t0, t0 + 4))
            pieces.append(vtr4())
            return pieces

        # global filler queue: (chunk_tag_or_None, closure).  Chunk pieces
        # and deferred output projections pop between attention iterations
        # at a self-balancing cadence; `reserve` pieces are held back so
        # later units never starve.
        fill_q = []
        # output staging: adjacent q-chunk pairs share one [DT, 2*SQ] tile
        # so each outp DMA moves 2KB rows (half the descriptor count);
        # pairs alternate between the sync and gpsimd queues.
        opairs = {}

        def pop_one():
            if fill_q:
                fill_q.pop(0)[1]()

        def emit_attn_unit(b, c, reserve=0, last=False):
            # returns tail closures (the row-parallel output projection) to
            # be deferred into later units' iteration loops
            qT, kT, vT, v_sb = qkv_tiles[b]
            J = (c + 1) * (SQ // KT)  # causal kv tiles for this chunk
            ctx_ps = acc_psum.tile([2 * E, SQ], F32, name=f"ctx_{b}_{c}", tag="ctx")
            denb = den_psum.tile([KT, SQ], F32, name=f"den_{b}_{c}", tag="den")
            def emit_denpv(idx, j, ex, cut):
                # denominator rides PE: ones.T @ ex accumulates the
                # per-column sums, already broadcast over partitions.
                # start/stop key on EMISSION order (idx), not kv index.
                for h in range(HL):
                    nc.tensor.matmul(
                        denb[h * E:(h + 1) * E, cut:SQ],
                        ones_sb[:, h * E:(h + 1) * E],
                        ex[:, h, cut:SQ],
                        start=(idx == 0),
                        stop=(idx == J - 1),
                        skip_group_check=True,
                    )
                for h in range(HL):
                    nc.tensor.matmul(
                        ctx_ps[h * E:(h + 1) * E, cut:SQ],
                        v_sb[:, j, h * E:(h + 1) * E],
                        ex[:, h, cut:SQ],
                        start=(idx == 0),
                        stop=(idx == J - 1),
                        skip_group_check=True,
                    )

            # diagonal kv tiles (small, exp/mask-paced) run FIRST while the
            # filler queue is full; the dense full-width tiles close the
            # unit back-to-back once fillers thin out
            js = list(range(max(0, J - 4), J)) + list(range(0, max(0, J - 4)))
            # kv tiles are processed in MACRO pairs with den/PV lagging by
            # one macro: the PE sees [sc,sc][den,pv,den,pv][fillers] with
            # only two tile-config switches (row<->col) per macro, the
            # second block of each kind paying no reconfigure penalty, and
            # exp (scalar) + mask (vector) a full macro off the critical
            # path.
            pending = []

            def emit_sc(idx, j):
                rdiag = j - (c * (SQ // KT))
                # columns [0, cut) of this q-chunk are fully masked for
                # diagonal kv tiles -- skip them everywhere
                cut = KT * rdiag if rdiag > 0 else 0
                n = SQ - cut
                sc = sc_psum.tile([KT, 2, SQ], F32, name=f"sc_{b}_{c}_{j}", tag="sc")
                ex = ex_pool.tile([KT, 2, SQ], BF16, name=f"ex_{b}_{c}_{j}", tag="ex")
                for h in range(HL):
                    nc.tensor.matmul(
                        sc[:, h, 0:n],
                        kT[h * E:(h + 1) * E, j * KT:(j + 1) * KT],
                        qT[h * E:(h + 1) * E, c * SQ + cut:(c + 1) * SQ],
                        start=True,
                        stop=True,
                    )
                nc.scalar.activation(
                    ex[:, :, cut:SQ], sc[:, :, 0:n], EXP, scale=0.125
                )
                if rdiag >= 0:
                    nc.vector.tensor_mul(
                        ex[:, :, cut:SQ],
                        ex[:, :, cut:SQ],
                        masks_sb[:, rdiag, :, cut:SQ],
                    )
                pending.append((idx, j, ex, cut))

            for idx, j in enumerate(js):
                pop_one()
                emit_sc(idx, j)
                while len(pending) > 2:
                    emit_denpv(*pending.pop(0))
                # self-balancing filler cadence: spread the queue (minus
                # the held-back reserve) evenly over remaining iterations
                quota = -(-max(0, len(fill_q) - reserve) // (J - idx)) - 1
                for _ in range(quota):
                    pop_one()
            for p in pending:
                emit_denpv(*p)

            recb = misc_pool.tile([KT, SQ], F32, name=f"rec_{b}_{c}", tag="recb")
            nc.vector.reciprocal_approx_fast(recb[:], denb[:])
            ctx_sb = misc_pool.tile(
                [2 * E, SQ], BF16, name=f"ctxsb_{b}_{c}", tag="ctxsb", bufs=3
            )
            nc.vector.tensor_mul(ctx_sb[:], ctx_ps[:], recb[:])

            # ---- row-parallel output projection (partial), deferred ----
            # These pieces pop during later units.  When they land in an
            # ACT-idle phase (after a c==3 unit, or the final flush), the
            # PSUM->SBUF copy goes to the scalar engine so the vector
            # engine's copy backlog doesn't stall the mm_psum rotation;
            # the final flush also borrows the (now idle) score banks.
            def oproj_piece(o, scalar_copy=False, use_sc_psum=False):
                def go():
                    pool = sc_psum if use_sc_psum else mm_psum
                    tag = "sc" if use_sc_psum else "mm"
                    ops = pool.tile([DT, SQ], F32, name=f"op_{b}_{c}_{o}", tag=tag)
                    for h in range(2):
                        nc.tensor.matmul(
                            ops[h * E:(h + 1) * E, :],
                            wo_sb[:, o * DT + h * E:o * DT + (h + 1) * E],
                            ctx_sb[:],
                            start=True, stop=True,
                            skip_group_check=True,
                        )
                    key = (b, c // 2, o)
                    ent = opairs.get(key)
                    if ent is None:
                        ent = opairs[key] = [
                            out_pool.tile(
                                [DT, 2, SQ], BF16, name=f"osb_{b}_{c // 2}_{o}",
                                tag="osb",
                            ),
                            0,
                        ]
                    osb = ent[0]
                    if scalar_copy:
                        nc.scalar.activation(
                            osb[:, c % 2, :], ops[:], mybir.ActivationFunctionType.Copy
                        )
                    else:
                        nc.vector.tensor_copy(osb[:, c % 2, :], ops[:])
                    ent[1] += 1
                    if ent[1] == 2:
                        del opairs[key]
                        if last:
                            # kernel tail: exp work is done, so the scalar
                            # queue is free to help drain the final outputs
                            q = (nc.sync, nc.gpsimd, nc.scalar)[o % 3]
                        else:
                            q = nc.sync if o % 2 == 0 else nc.gpsimd
                        q.dma_start(
                            outp[
                                o * DT:(o + 1) * DT,
                                b * S + (c // 2) * 2 * SQ: b * S + (c // 2 + 1) * 2 * SQ,
                            ],
                            osb[:],
                        )
                return go

            return [
                oproj_piece(
                    o,
                    scalar_copy=(last and o % 2 == 1),
                    use_sc_psum=(last and o % 2 == 1),
                )
                for o in range(D // DT)
            ]

        # software pipeline: the global queue runs two qkv chunks ahead of
        # the attention units, plus deferred output projections.  The last
        # batch's units are rotated so the final unit is a small one (J=4)
        # and the kernel tail stays dense.
        NU = B * NQ
        unit_order = [(b, c) for b in range(B) for c in range(NQ)]
        unit_order = unit_order[:-NQ] + unit_order[-NQ + 1:] + [unit_order[-NQ]]
        chunk_order = [(b, c) for b in range(B) for c in range(NQ)]

        for p in qkv_chunk_pieces(0, 0):
            p()
        nc.gpsimd.dma_start(masks_sb[:], masks[:])
        nc.sync.dma_start(wo_sb[:], wo[:])
        fill_q += [((0, 1), p) for p in qkv_chunk_pieces(0, 1)]

        for i, (b, c) in enumerate(unit_order):
            if i + 2 < NU:
                ch = chunk_order[i + 2]
                fill_q += [(ch, p) for p in qkv_chunk_pieces(*ch)]
            # guard: every chunk this unit reads must be emitted before
            # the unit's first score matmul
            while any(
                t is not None and t[0] == b and t[1] <= c for t, _ in fill_q
            ):
                pop_one()
            tail = emit_attn_unit(
                b, c, reserve=8 if i < NU - 2 else 0, last=(i == NU - 1)
            )
            fill_q += [(None, p) for p in tail]
        while fill_q:
            pop_one()

    nc.finalize()
    return nc


def _host_inputs(x, Wq, Wk, Wv, Wo):
    import ml_dtypes

    bf = ml_dtypes.bfloat16
    # [chunk, p, t, n]: per-chunk contiguous tiles of x^T
    xT = np.ascontiguousarray(
        x.reshape(B * NQ, SQ, ND, DT).transpose(0, 3, 2, 1)
    ).astype(bf)
    p = np.arange(KT)[:, None, None]
    rr = np.arange(NQ)[None, :, None]
    cc = np.arange(SQ)[None, None, :]
    masks = (cc >= KT * rr + p).astype(bf)
    # duplicated per head so the mask multiply is one [KT, 2, n] DVE op
    masks = np.ascontiguousarray(np.repeat(masks[:, :, None, :], 2, axis=2))
    in_maps = []
    for core in range(NCORES):
        hs = slice(core * HL, (core + 1) * HL)
        wq = Wq[hs].reshape(EL, D).T
        wk = Wk[hs].reshape(EL, D).T
        wv = Wv[hs].reshape(EL, D).T
        wqkv = np.ascontiguousarray(
            np.concatenate([wq, wk, wv], axis=1)
            .reshape(ND, DT, 3 * EL)
            .transpose(1, 0, 2)
        ).astype(bf)
        woL = np.ascontiguousarray(
            Wo[:, core * EL:(core + 1) * EL].T
        ).astype(bf)
        in_maps.append({"xT": xT, "wqkv": wqkv, "wo": woL, "masks": masks})
    return in_maps


def kernel(x, Wq, Wk, Wv, Wo):
    global LAST_RESULTS
    x, Wq, Wk, Wv, Wo = (np.asarray(a, dtype=np.float32) for a in (x, Wq, Wk, Wv, Wo))
    nc = build()
    in_maps = _host_inputs(x, Wq, Wk, Wv, Wo)
    import os
    res = run_bass_kernel_spmd(
        nc, in_maps, list(range(NCORES)),
        trace=bool(os.environ.get("BASS_KERNEL_TRACE")),
    )
    LAST_RESULTS = res
    acc = np.zeros((D, B * S), np.float32)
    for rmap in res.results:
        acc += rmap["outp"]
    return np.ascontiguousarray(acc.T).reshape(B, S, D)


if __name__ == "__main__":
    rng = np.random.default_rng(0)
    scale = 1.0 / np.sqrt(D)
    x = rng.standard_normal((B, S, D), dtype=np.float32)
    Wq = rng.standard_normal((H, E, D), dtype=np.float32) * scale
    Wk = rng.standard_normal((H, E, D), dtype=np.float32) * scale
    Wv = rng.standard_normal((H, E, D), dtype=np.float32) * scale
    Wo = rng.standard_normal((D, D), dtype=np.float32) * scale
    out = kernel(x, Wq, Wk, Wv, Wo)
    print(out.shape, out.dtype, float(np.abs(out).max()))



# revision 32
# speedup vs baseline: 1.3193x; 1.0216x over previous
"""Multi-head causal attention (B=4, S=2048, D=1024, H=16) on 8 TRN2 cores.

Sharding: tensor-parallel over heads (2 heads/core), proj_out row-parallel
with the cross-core reduction done host-side during unsharding.

Per-core kernel layout (all contractions on the SBUF partition axis):
  xT      (1024 d, 8192 tok)   host-pretransposed activations (shared input)
  qT/kT   (128 e2, 2048 s)     per batch; e2 = 2 heads x 64
  scoresT (128 sk, 512 sq)     kv-major scores -> exp -> PV matmul directly
  denom   ones-matmul broadcast of the per-column sums of exp(scores)
  ctxT    (128 e2, 512 sq)     normalized, fed straight into row-parallel Wo
  outp    (1024 o, 8192 tok)   per-core partial; host sums over cores

Head pairs run concurrently on the PE via automatic tile_position (row
tiles for the 64-contraction score matmuls, col tiles for the 64-wide
den/PV matmuls).  Pipeline: warmup matmuls keep the HAM clock-gate warm
through the initial DMA; qkv projection + deferred output-projection
work is interleaved at fine grain between attention iterations so the
PE never idles long enough to re-throttle.

v2: den/PV lag scores by 2 iterations (exp+mask off the PE critical
path), a filler precedes each score pair (LDWEIGHTS cover), output
DMAs move 2-chunk 2KB rows split across sync+gpsimd queues, warmup
deps issue first on the vector queue.
"""

import sys

if "/opt/trn_rl_repo" not in sys.path:
    sys.path.insert(0, "/opt/trn_rl_repo")

from contextlib import ExitStack

import numpy as np

import concourse.bass as bass
import concourse.bacc as bacc
import concourse.mybir as mybir
import concourse.tile as tile
from concourse.bass_utils import run_bass_kernel_spmd
from concourse.masks import make_identity

B, S, D, H, E = 4, 2048, 1024, 16, 64
NCORES = 8
HL = H // NCORES          # heads per core = 2
EL = HL * E               # local feature width = 128
SQ = 512                  # query chunk (matmul moving dim)
NQ = S // SQ              # 4
KT = 128                  # kv tile (contraction tile)
DT = 128                  # d-model contraction tile
ND = D // DT              # 8
N_WARM = 22               # HAM warmup matmuls during initial DMA
F32 = mybir.dt.float32
BF16 = mybir.dt.bfloat16
EXP = mybir.ActivationFunctionType.Exp

LAST_RESULTS = None


def build():
    nc = bacc.Bacc()
    xT = nc.declare_dram_parameter("xT", [B * NQ, DT, ND, SQ], BF16, isOutput=False)
    wqkv = nc.declare_dram_parameter("wqkv", [DT, ND, 3 * EL], BF16, isOutput=False)
    wo = nc.declare_dram_parameter("wo", [EL, D], BF16, isOutput=False)
    masks = nc.declare_dram_parameter("masks", [KT, NQ, 2, SQ], BF16, isOutput=False)
    outp = nc.declare_dram_parameter("outp", [D, B * S], BF16, isOutput=True)

    with tile.TileContext(nc) as tc, ExitStack() as ctx:
        consts = ctx.enter_context(tc.tile_pool(name="consts", bufs=1))
        xt_pool = ctx.enter_context(tc.tile_pool(name="xt", bufs=3))
        qk_pool = ctx.enter_context(tc.tile_pool(name="qk", bufs=2))
        ex_pool = ctx.enter_context(tc.tile_pool(name="ex", bufs=8))
        misc_pool = ctx.enter_context(tc.tile_pool(name="misc", bufs=2))
        out_pool = ctx.enter_context(tc.tile_pool(name="outsb", bufs=16))
        mm_psum = ctx.enter_context(tc.tile_pool(name="mmps", bufs=2, space="PSUM"))
        sc_psum = ctx.enter_context(tc.tile_pool(name="scps", bufs=2, space="PSUM"))
        acc_psum = ctx.enter_context(tc.tile_pool(name="accps", bufs=1, space="PSUM"))
        den_psum = ctx.enter_context(tc.tile_pool(name="denps", bufs=1, space="PSUM"))

        # DMA queue plan for the startup window: the sync (HWDGE) queue
        # carries the first half of wqkv, then the first x chunk's low
        # half, then the rest of wqkv -- so the t<4 projection matmuls
        # (and the wqkv-fed warmup) can start as early as possible.
        # masks/wo are only needed a few us in so they load later.
        wqkv_sb = consts.tile([DT, ND, 3 * EL], BF16)
        nc.sync.dma_start(wqkv_sb[:, 0:ND // 2, :], wqkv[:, 0:ND // 2, :])
        masks_sb = consts.tile([KT, NQ, 2, SQ], BF16)
        wo_sb = consts.tile([EL, D], BF16)
        ones_sb = consts.tile([DT, DT], BF16)
        nc.vector.memset(ones_sb[:], 1.0)

        qkv_tiles = {}
        xt_tiles = {}

        def load_chunk(b, c):
            xt8 = xt_pool.tile([DT, ND, SQ], BF16, name=f"xt_{b}_{c}", tag="xt")
            # x is host-prechunked to [chunk, p, t, n] so each load is one
            # contiguous 8KB-per-partition transfer; split across HWDGE
            # (sync) and SWDGE (gpsimd) so the two halves stream in
            # parallel and the t<4 matmuls start after half has landed
            half = ND // 2
            u = b * NQ + c
            nc.sync.dma_start(xt8[:, 0:half, :], xT[u, :, 0:half, :])
            nc.gpsimd.dma_start(xt8[:, half:ND, :], xT[u, :, half:ND, :])
            xt_tiles[(b, c)] = xt8
            return xt8

        load_chunk(0, 0)
        nc.sync.dma_start(wqkv_sb[:, ND // 2:ND, :], wqkv[:, ND // 2:ND, :])

        # HAM warmup: keep the PE busy from the earliest possible moment
        # (gated only on the ones memset) until the first x chunk lands
        # (~17us) so the clock-gate is at 8/8 when real work issues.
        warm_src = consts.tile([DT, SQ], BF16)
        nc.vector.memset(warm_src[:], 0.0)
        warm_ps = mm_psum.tile([DT, SQ], F32, name="warm_ps", tag="mm")
        for _ in range(N_WARM):
            nc.tensor.matmul(warm_ps[:], ones_sb[:], warm_src[:], start=True, stop=True)

        ident = consts.tile([DT, DT], BF16)
        make_identity(nc, ident[:])

        def qkv_chunk_pieces(b, c):
            # returns fine-grained filler closures; each emits a small piece
            # of the qkv work for chunk (b, c) so it can be sprinkled between
            # attention iterations (engine FIFOs are strict in-order)
            if c == 0:
                qT = qk_pool.tile([EL, S], BF16, name=f"qT_{b}", tag="qT")
                kT = qk_pool.tile([EL, S], BF16, name=f"kT_{b}", tag="kT")
                vT = qk_pool.tile([EL, S], BF16, name=f"vT_{b}", tag="vT", bufs=1)
                v_sb = qk_pool.tile([KT, S // KT, EL], BF16, name=f"v_{b}", tag="v")
                qkv_tiles[b] = (qT, kT, vT, v_sb)
            qT, kT, vT, v_sb = qkv_tiles[b]
            xt8 = xt_tiles.get((b, c))
            if xt8 is None:
                xt8 = load_chunk(b, c)

            psums = {}

            def proj_piece(dest, col0, t0, t1):
                # emitted as col-tile pairs (two concurrent [128,64] tiles)
                # so fillers share the den/PV tile config -- avoids the
                # ~110ns PE array-reconfigure penalty per switch
                def go():
                    if t0 == 0:
                        psums[col0] = mm_psum.tile(
                            [EL, SQ], F32, name=f"qkv_ps_{b}_{c}_{col0}", tag="mm"
                        )
                    ps = psums[col0]
                    for t in range(t0, t1):
                        for h in range(2):
                            nc.tensor.matmul(
                                ps[h * E:(h + 1) * E, :],
                                wqkv_sb[:, t, col0 + h * E:col0 + (h + 1) * E],
                                xt8[:, t, :],
                                start=(t == 0),
                                stop=(t == ND - 1),
                                skip_group_check=True,
                            )
                    if t1 == ND:
                        nc.vector.tensor_copy(dest[:, c * SQ:(c + 1) * SQ], ps[:])
                return go

            def vtr4():
                # all four transposes in one piece: transpose-mode is its
                # own PE tile config, so batching them pays the reconfigure
                # penalty once per chunk instead of per pair
                def go():
                    for j in range(4 * c, 4 * c + 4):
                        vt_ps = mm_psum.tile([KT, KT], BF16, name=f"vt_ps_{b}_{j}", tag="mm")
                        nc.tensor.transpose(vt_ps[:], vT[:, j * KT:(j + 1) * KT], ident[:])
                        nc.vector.tensor_copy(v_sb[:, j, :], vt_ps[:])
                return go

            pieces = []
            for col0, dest in ((0, qT), (EL, kT), (2 * EL, vT)):
                for t0 in range(0, ND, 4):
                    pieces.append(proj_piece(dest, col0, t0, t0 + 4))
            pieces.append(vtr4())
            return pieces

        # global filler queue: (chunk_tag_or_None, closure).  Chunk pieces
        # and deferred output projections pop between attention iterations
        # at a self-balancing cadence; `reserve` pieces are held back so
        # later units never starve.
        fill_q = []
        # output staging: adjacent q-chunk pairs share one [DT, 2*SQ] tile
        # so each outp DMA moves 2KB rows (half the descriptor count);
        # pairs alternate between the sync and gpsimd queues.
        opairs = {}

        def pop_one():
            if fill_q:
                fill_q.pop(0)[1]()

        def emit_attn_unit(b, c, reserve=0, last=False):
            # returns tail closures (the row-parallel output projection) to
            # be deferred into later units' iteration loops
            qT, kT, vT, v_sb = qkv_tiles[b]
            J = (c + 1) * (SQ // KT)  # causal kv tiles for this chunk
            ctx_ps = acc_psum.tile([2 * E, SQ], F32, name=f"ctx_{b}_{c}", tag="ctx")
            denb = den_psum.tile([KT, SQ], F32, name=f"den_{b}_{c}", tag="den")
            def emit_denpv(idx, j, ex, cut):
                # denominator rides PE: ones.T @ ex accumulates the
                # per-column sums, already broadcast over partitions.
                # start/stop key on EMISSION order (idx), not kv index.
                for h in range(HL):
                    nc.tensor.matmul(
                        denb[h * E:(h + 1) * E, cut:SQ],
                        ones_sb[:, h * E:(h + 1) * E],
                        ex[:, h, cut:SQ],
                        start=(idx == 0),
                        stop=(idx == J - 1),
                        skip_group_check=True,
                    )
                for h in range(HL):
                    nc.tensor.matmul(
                        ctx_ps[h * E:(h + 1) * E, cut:SQ],
                        v_sb[:, j, h * E:(h + 1) * E],
                        ex[:, h, cut:SQ],
                        start=(idx == 0),
                        stop=(idx == J - 1),
                        skip_group_check=True,
                    )

            # diagonal kv tiles (small, exp/mask-paced) run FIRST while the
            # filler queue is full; the dense full-width tiles close the
            # unit back-to-back once fillers thin out
            js = list(range(max(0, J - 4), J)) + list(range(0, max(0, J - 4)))
            # kv tiles are processed in MACRO pairs with den/PV lagging by
            # one macro: the PE sees [sc,sc][den,pv,den,pv][fillers] with
            # only two tile-config switches (row<->col) per macro, the
            # second block of each kind paying no reconfigure penalty, and
            # exp (scalar) + mask (vector) a full macro off the critical
            # path.
            pending = []

            def emit_sc(idx, j):
                rdiag = j - (c * (SQ // KT))
                # columns [0, cut) of this q-chunk are fully masked for
                # diagonal kv tiles -- skip them everywhere
                cut = KT * rdiag if rdiag > 0 else 0
                n = SQ - cut
                sc = sc_psum.tile([KT, 2, SQ], F32, name=f"sc_{b}_{c}_{j}", tag="sc")
                ex = ex_pool.tile([KT, 2, SQ], BF16, name=f"ex_{b}_{c}_{j}", tag="ex")
                for h in range(HL):
                    nc.tensor.matmul(
                        sc[:, h, 0:n],
                        kT[h * E:(h + 1) * E, j * KT:(j + 1) * KT],
                        qT[h * E:(h + 1) * E, c * SQ + cut:(c + 1) * SQ],
                        start=True,
                        stop=True,
                    )
                nc.scalar.activation(
                    ex[:, :, cut:SQ], sc[:, :, 0:n], EXP, scale=0.125
                )
                if rdiag >= 0:
                    nc.vector.tensor_mul(
                        ex[:, :, cut:SQ],
                        ex[:, :, cut:SQ],
                        masks_sb[:, rdiag, :, cut:SQ],
                    )
                pending.append((idx, j, ex, cut))

            for idx, j in enumerate(js):
                pop_one()
                emit_sc(idx, j)
                # on the unit's last iteration, drain the den/PV pipeline
                # right away (with filler cover for the fresh exps) so the
                # recb/ctx chain starts ~2 iterations earlier and the next
                # unit's den never waits on the acc/den psum banks
                lag = 0 if idx == J - 1 else 2
                if lag == 0:
                    pop_one()
                    pop_one()
                while len(pending) > lag:
                    emit_denpv(*pending.pop(0))
                # self-balancing filler cadence: spread the queue (minus
                # the held-back reserve) evenly over remaining iterations
                quota = -(-max(0, len(fill_q) - reserve) // (J - idx)) - 1
                for _ in range(quota):
                    pop_one()
            for p in pending:
                emit_denpv(*p)

            recb = misc_pool.tile([KT, SQ], F32, name=f"rec_{b}_{c}", tag="recb")
            nc.vector.reciprocal_approx_fast(recb[:], denb[:])
            ctx_sb = misc_pool.tile(
                [2 * E, SQ], BF16, name=f"ctxsb_{b}_{c}", tag="ctxsb", bufs=3
            )
            nc.vector.tensor_mul(ctx_sb[:], ctx_ps[:], recb[:])

            # ---- row-parallel output projection (partial), deferred ----
            # These pieces pop during later units.  When they land in an
            # ACT-idle phase (after a c==3 unit, or the final flush), the
            # PSUM->SBUF copy goes to the scalar engine so the vector
            # engine's copy backlog doesn't stall the mm_psum rotation;
            # the final flush also borrows the (now idle) score banks.
            def oproj_piece(o, scalar_copy=False, use_sc_psum=False):
                def go():
                    pool = sc_psum if use_sc_psum else mm_psum
                    tag = "sc" if use_sc_psum else "mm"
                    ops = pool.tile([DT, SQ], F32, name=f"op_{b}_{c}_{o}", tag=tag)
                    for h in range(2):
                        nc.tensor.matmul(
                            ops[h * E:(h + 1) * E, :],
                            wo_sb[:, o * DT + h * E:o * DT + (h + 1) * E],
                            ctx_sb[:],
                            start=True, stop=True,
                            skip_group_check=True,
                        )
                    key = (b, c // 2, o)
                    ent = opairs.get(key)
                    if ent is None:
                        ent = opairs[key] = [
                            out_pool.tile(
                                [DT, 2, SQ], BF16, name=f"osb_{b}_{c // 2}_{o}",
                                tag="osb",
                            ),
                            0,
                        ]
                    osb = ent[0]
                    if scalar_copy:
                        nc.scalar.activation(
                            osb[:, c % 2, :], ops[:], mybir.ActivationFunctionType.Copy
                        )
                    else:
                        nc.vector.tensor_copy(osb[:, c % 2, :], ops[:])
                    ent[1] += 1
                    if ent[1] == 2:
                        del opairs[key]
                        if last:
                            # kernel tail: exp work is done, so the scalar
                            # queue is free to help drain the final outputs
                            q = (nc.sync, nc.gpsimd, nc.scalar)[o % 3]
                        else:
                            q = nc.sync if o % 2 == 0 else nc.gpsimd
                        q.dma_start(
                            outp[
                                o * DT:(o + 1) * DT,
                                b * S + (c // 2) * 2 * SQ: b * S + (c // 2 + 1) * 2 * SQ,
                            ],
                            osb[:],
                        )
                return go

            return [
                oproj_piece(
                    o,
                    scalar_copy=(last and o % 2 == 1),
                    use_sc_psum=(last and o % 2 == 1),
                )
                for o in range(D // DT)
            ]

        # software pipeline: the global queue runs two qkv chunks ahead of
        # the attention units, plus deferred output projections.  The last
        # batch's units are rotated so the final unit is a small one (J=4)
        # and the kernel tail stays dense.
        NU = B * NQ
        unit_order = [(b, c) for b in range(B) for c in range(NQ)]
        unit_order = unit_order[:-NQ] + unit_order[-NQ + 1:] + [unit_order[-NQ]]
        chunk_order = [(b, c) for b in range(B) for c in range(NQ)]

        for p in qkv_chunk_pieces(0, 0):
            p()
        nc.gpsimd.dma_start(masks_sb[:], masks[:])
        nc.sync.dma_start(wo_sb[:], wo[:])
        fill_q += [((0, 1), p) for p in qkv_chunk_pieces(0, 1)]

        for i, (b, c) in enumerate(unit_order):
            if i + 2 < NU:
                ch = chunk_order[i + 2]
                fill_q += [(ch, p) for p in qkv_chunk_pieces(*ch)]
            # guard: every chunk this unit reads must be emitted before
            # the unit's first score matmul
            while any(
                t is not None and t[0] == b and t[1] <= c for t, _ in fill_q
            ):
                pop_one()
            tail = emit_attn_unit(
                b, c, reserve=8 if i < NU - 2 else 0, last=(i == NU - 1)
            )
            fill_q += [(None, p) for p in tail]
        while fill_q:
            pop_one()

    nc.finalize()
    return nc


def _host_inputs(x, Wq, Wk, Wv, Wo):
    import ml_dtypes

    bf = ml_dtypes.bfloat16
    # [chunk, p, t, n]: per-chunk contiguous tiles of x^T
    xT = np.ascontiguousarray(
        x.reshape(B * NQ, SQ, ND, DT).transpose(0, 3, 2, 1)
    ).astype(bf)
    p = np.arange(KT)[:, None, None]
    rr = np.arange(NQ)[None, :, None]
    cc = np.arange(SQ)[None, None, :]
    masks = (cc >= KT * rr + p).astype(bf)
    # duplicated per head so the mask multiply is one [KT, 2, n] DVE op
    masks = np.ascontiguousarray(np.repeat(masks[:, :, None, :], 2, axis=2))
    in_maps = []
    for core in range(NCORES):
        hs = slice(core * HL, (core + 1) * HL)
        wq = Wq[hs].reshape(EL, D).T
        wk = Wk[hs].reshape(EL, D).T
        wv = Wv[hs].reshape(EL, D).T
        wqkv = np.ascontiguousarray(
            np.concatenate([wq, wk, wv], axis=1)
            .reshape(ND, DT, 3 * EL)
            .transpose(1, 0, 2)
        ).astype(bf)
        woL = np.ascontiguousarray(
            Wo[:, core * EL:(core + 1) * EL].T
        ).astype(bf)
        in_maps.append({"xT": xT, "wqkv": wqkv, "wo": woL, "masks": masks})
    return in_maps


def kernel(x, Wq, Wk, Wv, Wo):
    global LAST_RESULTS
    x, Wq, Wk, Wv, Wo = (np.asarray(a, dtype=np.float32) for a in (x, Wq, Wk, Wv, Wo))
    nc = build()
    in_maps = _host_inputs(x, Wq, Wk, Wv, Wo)
    import os
    res = run_bass_kernel_spmd(
        nc, in_maps, list(range(NCORES)),
        trace=bool(os.environ.get("BASS_KERNEL_TRACE")),
    )
    LAST_RESULTS = res
    acc = np.zeros((D, B * S), np.float32)
    for rmap in res.results:
        acc += rmap["outp"]
    return np.ascontiguousarray(acc.T).reshape(B, S, D)


if __name__ == "__main__":
    rng = np.random.default_rng(0)
    scale = 1.0 / np.sqrt(D)
    x = rng.standard_normal((B, S, D), dtype=np.float32)
    Wq = rng.standard_normal((H, E, D), dtype=np.float32) * scale
    Wk = rng.standard_normal((H, E, D), dtype=np.float32) * scale
    Wv = rng.standard_normal((H, E, D), dtype=np.float32) * scale
    Wo = rng.standard_normal((D, D), dtype=np.float32) * scale
    out = kernel(x, Wq, Wk, Wv, Wo)
    print(out.shape, out.dtype, float(np.abs(out).max()))



# revision 37
# speedup vs baseline: 1.3422x; 1.0173x over previous
"""Multi-head causal attention (B=4, S=2048, D=1024, H=16) on 8 TRN2 cores.

Sharding: tensor-parallel over heads (2 heads/core), proj_out row-parallel
with the cross-core reduction done host-side during unsharding.

Per-core kernel layout (all contractions on the SBUF partition axis):
  xT      (1024 d, 8192 tok)   host-pretransposed activations (shared input)
  qT/kT   (128 e2, 2048 s)     per batch; e2 = 2 heads x 64
  scoresT (128 sk, 512 sq)     kv-major scores -> exp -> PV matmul directly
  denom   ones-matmul broadcast of the per-column sums of exp(scores)
  ctxT    (128 e2, 512 sq)     normalized, fed straight into row-parallel Wo
  outp    (1024 o, 8192 tok)   per-core partial; host sums over cores

Head pairs run concurrently on the PE via automatic tile_position (row
tiles for the 64-contraction score matmuls, col tiles for the 64-wide
den/PV matmuls).  Pipeline: warmup matmuls keep the HAM clock-gate warm
through the initial DMA; qkv projection + deferred output-projection
work is interleaved at fine grain between attention iterations so the
PE never idles long enough to re-throttle.

v2: den/PV lag scores by 2 iterations (exp+mask off the PE critical
path), a filler precedes each score pair (LDWEIGHTS cover), output
DMAs move 2-chunk 2KB rows split across sync+gpsimd queues, warmup
deps issue first on the vector queue.
"""

import sys

if "/opt/trn_rl_repo" not in sys.path:
    sys.path.insert(0, "/opt/trn_rl_repo")

from contextlib import ExitStack

import numpy as np

import concourse.bass as bass
import concourse.bacc as bacc
import concourse.mybir as mybir
import concourse.tile as tile
from concourse.bass_utils import run_bass_kernel_spmd
from concourse.masks import make_identity

B, S, D, H, E = 4, 2048, 1024, 16, 64
NCORES = 8
HL = H // NCORES          # heads per core = 2
EL = HL * E               # local feature width = 128
SQ = 512                  # query chunk (matmul moving dim)
NQ = S // SQ              # 4
KT = 128                  # kv tile (contraction tile)
DT = 128                  # d-model contraction tile
ND = D // DT              # 8
N_WARM = 14               # HAM warmup matmuls during initial DMA
F32 = mybir.dt.float32
BF16 = mybir.dt.bfloat16
EXP = mybir.ActivationFunctionType.Exp

LAST_RESULTS = None


def build():
    nc = bacc.Bacc()
    xT = nc.declare_dram_parameter("xT", [B * NQ, DT, ND, SQ], BF16, isOutput=False)
    wqkv = nc.declare_dram_parameter("wqkv", [DT, ND, 3 * EL], BF16, isOutput=False)
    wo = nc.declare_dram_parameter("wo", [EL, D], BF16, isOutput=False)
    masks = nc.declare_dram_parameter("masks", [KT, NQ, 2, SQ], BF16, isOutput=False)
    outp = nc.declare_dram_parameter("outp", [D, B * S], BF16, isOutput=True)

    with tile.TileContext(nc) as tc, ExitStack() as ctx:
        consts = ctx.enter_context(tc.tile_pool(name="consts", bufs=1))
        xt_pool = ctx.enter_context(tc.tile_pool(name="xt", bufs=3))
        qk_pool = ctx.enter_context(tc.tile_pool(name="qk", bufs=2))
        ex_pool = ctx.enter_context(tc.tile_pool(name="ex", bufs=8))
        misc_pool = ctx.enter_context(tc.tile_pool(name="misc", bufs=2))
        out_pool = ctx.enter_context(tc.tile_pool(name="outsb", bufs=16))
        mm_psum = ctx.enter_context(tc.tile_pool(name="mmps", bufs=2, space="PSUM"))
        sc_psum = ctx.enter_context(tc.tile_pool(name="scps", bufs=2, space="PSUM"))
        acc_psum = ctx.enter_context(tc.tile_pool(name="accps", bufs=1, space="PSUM"))
        den_psum = ctx.enter_context(tc.tile_pool(name="denps", bufs=1, space="PSUM"))

        # DMA queue plan for the startup window: three queues in parallel
        # (sync/gpsimd carry the first x chunk halves, the scalar HWDGE
        # ring carries wqkv + the first mask row + wo) so the first
        # projection matmuls are gated on ~0.5MB per queue instead of a
        # serial 1.25MB.
        wqkv_sb = consts.tile([DT, ND, 3 * EL], BF16)
        nc.scalar.dma_start(wqkv_sb[:, 0:ND // 2, :], wqkv[:, 0:ND // 2, :])
        masks_sb = consts.tile([KT, NQ, 2, SQ], BF16)
        wo_sb = consts.tile([EL, D], BF16)
        ones_sb = consts.tile([DT, DT], BF16)
        nc.vector.memset(ones_sb[:], 1.0)

        qkv_tiles = {}
        xt_tiles = {}

        def load_chunk(b, c):
            xt8 = xt_pool.tile([DT, ND, SQ], BF16, name=f"xt_{b}_{c}", tag="xt")
            # x is host-prechunked to [chunk, p, t, n] so each load is one
            # contiguous 8KB-per-partition transfer; split across HWDGE
            # (sync) and SWDGE (gpsimd) so the two halves stream in
            # parallel and the t<4 matmuls start after half has landed
            half = ND // 2
            u = b * NQ + c
            nc.sync.dma_start(xt8[:, 0:half, :], xT[u, :, 0:half, :])
            nc.gpsimd.dma_start(xt8[:, half:ND, :], xT[u, :, half:ND, :])
            xt_tiles[(b, c)] = xt8
            return xt8

        load_chunk(0, 0)
        nc.scalar.dma_start(wqkv_sb[:, ND // 2:ND, :], wqkv[:, ND // 2:ND, :])
        nc.scalar.dma_start(masks_sb[:, 0:1, :, :], masks[:, 0:1, :, :])

        # HAM warmup: keep the PE busy from the earliest possible moment
        # (gated only on the ones memset) until the first x chunk lands
        # (~17us) so the clock-gate is at 8/8 when real work issues.
        warm_src = consts.tile([DT, SQ], BF16)
        nc.vector.memset(warm_src[:], 0.0)
        warm_ps = mm_psum.tile([DT, SQ], F32, name="warm_ps", tag="mm")
        for _ in range(N_WARM):
            nc.tensor.matmul(warm_ps[:], ones_sb[:], warm_src[:], start=True, stop=True)

        ident = consts.tile([DT, DT], BF16)
        make_identity(nc, ident[:])

        def qkv_chunk_pieces(b, c):
            # returns fine-grained filler closures; each emits a small piece
            # of the qkv work for chunk (b, c) so it can be sprinkled between
            # attention iterations (engine FIFOs are strict in-order)
            if c == 0:
                qT = qk_pool.tile([EL, S], BF16, name=f"qT_{b}", tag="qT")
                kT = qk_pool.tile([EL, S], BF16, name=f"kT_{b}", tag="kT")
                vT = qk_pool.tile([EL, S], BF16, name=f"vT_{b}", tag="vT", bufs=1)
                v_sb = qk_pool.tile([KT, S // KT, EL], BF16, name=f"v_{b}", tag="v")
                qkv_tiles[b] = (qT, kT, vT, v_sb)
            qT, kT, vT, v_sb = qkv_tiles[b]
            xt8 = xt_tiles.get((b, c))
            if xt8 is None:
                xt8 = load_chunk(b, c)

            psums = {}

            def proj_piece(dest, col0, t0, t1):
                # emitted as col-tile pairs (two concurrent [128,64] tiles)
                # so fillers share the den/PV tile config -- avoids the
                # ~110ns PE array-reconfigure penalty per switch
                def go():
                    if t0 == 0:
                        psums[col0] = mm_psum.tile(
                            [EL, SQ], F32, name=f"qkv_ps_{b}_{c}_{col0}", tag="mm"
                        )
                    ps = psums[col0]
                    for t in range(t0, t1):
                        for h in range(2):
                            nc.tensor.matmul(
                                ps[h * E:(h + 1) * E, :],
                                wqkv_sb[:, t, col0 + h * E:col0 + (h + 1) * E],
                                xt8[:, t, :],
                                start=(t == 0),
                                stop=(t == ND - 1),
                                skip_group_check=True,
                            )
                    if t1 == ND:
                        nc.vector.tensor_copy(dest[:, c * SQ:(c + 1) * SQ], ps[:])
                return go

            def vtr4():
                # all four transposes in one piece: transpose-mode is its
                # own PE tile config, so batching them pays the reconfigure
                # penalty once per chunk instead of per pair
                def go():
                    for j in range(4 * c, 4 * c + 4):
                        vt_ps = mm_psum.tile([KT, KT], BF16, name=f"vt_ps_{b}_{j}", tag="mm")
                        nc.tensor.transpose(vt_ps[:], vT[:, j * KT:(j + 1) * KT], ident[:])
                        nc.vector.tensor_copy(v_sb[:, j, :], vt_ps[:])
                return go

            pieces = []
            for col0, dest in ((0, qT), (EL, kT), (2 * EL, vT)):
                for t0 in range(0, ND, 4):
                    pieces.append(proj_piece(dest, col0, t0, t0 + 4))
            pieces.append(vtr4())
            return pieces

        # global filler queue: (chunk_tag_or_None, closure).  Chunk pieces
        # and deferred output projections pop between attention iterations
        # at a self-balancing cadence; `reserve` pieces are held back so
        # later units never starve.
        fill_q = []
        # output staging: adjacent q-chunk pairs share one [DT, 2*SQ] tile
        # so each outp DMA moves 2KB rows (half the descriptor count);
        # pairs alternate between the sync and gpsimd queues.
        opairs = {}

        def pop_one():
            if fill_q:
                fill_q.pop(0)[1]()

        def emit_attn_unit(b, c, reserve=0, last=False):
            # returns tail closures (the row-parallel output projection) to
            # be deferred into later units' iteration loops
            qT, kT, vT, v_sb = qkv_tiles[b]
            J = (c + 1) * (SQ // KT)  # causal kv tiles for this chunk
            ctx_ps = acc_psum.tile([2 * E, SQ], F32, name=f"ctx_{b}_{c}", tag="ctx")
            denb = den_psum.tile([KT, SQ], F32, name=f"den_{b}_{c}", tag="den")
            def emit_denpv(idx, j, ex, cut):
                # denominator rides PE: ones.T @ ex accumulates the
                # per-column sums, already broadcast over partitions.
                # start/stop key on EMISSION order (idx), not kv index.
                for h in range(HL):
                    nc.tensor.matmul(
                        denb[h * E:(h + 1) * E, cut:SQ],
                        ones_sb[:, h * E:(h + 1) * E],
                        ex[:, h, cut:SQ],
                        start=(idx == 0),
                        stop=(idx == J - 1),
                        skip_group_check=True,
                    )
                for h in range(HL):
                    nc.tensor.matmul(
                        ctx_ps[h * E:(h + 1) * E, cut:SQ],
                        v_sb[:, j, h * E:(h + 1) * E],
                        ex[:, h, cut:SQ],
                        start=(idx == 0),
                        stop=(idx == J - 1),
                        skip_group_check=True,
                    )

            # diagonal kv tiles (small, exp/mask-paced) run FIRST while the
            # filler queue is full; the dense full-width tiles close the
            # unit back-to-back once fillers thin out
            js = list(range(max(0, J - 4), J)) + list(range(0, max(0, J - 4)))
            # kv tiles are processed in MACRO pairs with den/PV lagging by
            # one macro: the PE sees [sc,sc][den,pv,den,pv][fillers] with
            # only two tile-config switches (row<->col) per macro, the
            # second block of each kind paying no reconfigure penalty, and
            # exp (scalar) + mask (vector) a full macro off the critical
            # path.
            pending = []

            def emit_sc(idx, j):
                rdiag = j - (c * (SQ // KT))
                # columns [0, cut) of this q-chunk are fully masked for
                # diagonal kv tiles -- skip them everywhere
                cut = KT * rdiag if rdiag > 0 else 0
                n = SQ - cut
                sc = sc_psum.tile([KT, 2, SQ], F32, name=f"sc_{b}_{c}_{j}", tag="sc")
                ex = ex_pool.tile([KT, 2, SQ], BF16, name=f"ex_{b}_{c}_{j}", tag="ex")
                for h in range(HL):
                    nc.tensor.matmul(
                        sc[:, h, 0:n],
                        kT[h * E:(h + 1) * E, j * KT:(j + 1) * KT],
                        qT[h * E:(h + 1) * E, c * SQ + cut:(c + 1) * SQ],
                        start=True,
                        stop=True,
                    )
                nc.scalar.activation(
                    ex[:, :, cut:SQ], sc[:, :, 0:n], EXP, scale=0.125
                )
                if rdiag >= 0:
                    nc.vector.tensor_mul(
                        ex[:, :, cut:SQ],
                        ex[:, :, cut:SQ],
                        masks_sb[:, rdiag, :, cut:SQ],
                    )
                pending.append((idx, j, ex, cut))

            for idx, j in enumerate(js):
                pop_one()
                emit_sc(idx, j)
                # on the unit's last iteration, drain the den/PV pipeline
                # right away (with filler cover for the fresh exps) so the
                # recb/ctx chain starts ~2 iterations earlier and the next
                # unit's den never waits on the acc/den psum banks
                lag = 0 if idx == J - 1 else 2
                if lag == 0:
                    pop_one()
                    pop_one()
                while len(pending) > lag:
                    emit_denpv(*pending.pop(0))
                # self-balancing filler cadence: spread the queue (minus
                # the held-back reserve) evenly over remaining iterations
                quota = -(-max(0, len(fill_q) - reserve) // (J - idx)) - 1
                for _ in range(quota):
                    pop_one()
            for p in pending:
                emit_denpv(*p)

            recb = misc_pool.tile([KT, SQ], F32, name=f"rec_{b}_{c}", tag="recb")
            nc.vector.reciprocal_approx_fast(recb[:], denb[:])
            ctx_sb = misc_pool.tile(
                [2 * E, SQ], BF16, name=f"ctxsb_{b}_{c}", tag="ctxsb", bufs=3
            )
            nc.vector.tensor_mul(ctx_sb[:], ctx_ps[:], recb[:])

            # ---- row-parallel output projection (partial), deferred ----
            # These pieces pop during later units.  When they land in an
            # ACT-idle phase (after a c==3 unit, or the final flush), the
            # PSUM->SBUF copy goes to the scalar engine so the vector
            # engine's copy backlog doesn't stall the mm_psum rotation;
            # the final flush also borrows the (now idle) score banks.
            def oproj_piece(o, scalar_copy=False, use_sc_psum=False):
                def go():
                    pool = sc_psum if use_sc_psum else mm_psum
                    tag = "sc" if use_sc_psum else "mm"
                    ops = pool.tile([DT, SQ], F32, name=f"op_{b}_{c}_{o}", tag=tag)
                    for h in range(2):
                        nc.tensor.matmul(
                            ops[h * E:(h + 1) * E, :],
                            wo_sb[:, o * DT + h * E:o * DT + (h + 1) * E],
                            ctx_sb[:],
                            start=True, stop=True,
                            skip_group_check=True,
                        )
                    # the rotated last unit (B-1, 0) runs ~12 units after
                    # its pair partner (B-1, 1): fire those halves as
                    # singles so the partner's data doesn't sit in SBUF
                    # until the kernel tail
                    if b == B - 1 and c < 2:
                        osb = out_pool.tile(
                            [DT, SQ], BF16, name=f"osbS_{b}_{c}_{o}", tag="osb"
                        )
                        if scalar_copy:
                            nc.scalar.activation(
                                osb[:], ops[:], mybir.ActivationFunctionType.Copy
                            )
                        else:
                            nc.vector.tensor_copy(osb[:], ops[:])
                        if last:
                            # kernel tail: exp work is done, so the scalar
                            # queue is free to help drain the final outputs
                            q = (nc.sync, nc.gpsimd, nc.scalar)[o % 3]
                        else:
                            q = nc.sync if o % 2 == 0 else nc.gpsimd
                        q.dma_start(
                            outp[
                                o * DT:(o + 1) * DT,
                                b * S + c * SQ: b * S + (c + 1) * SQ,
                            ],
                            osb[:],
                        )
                        return
                    key = (b, c // 2, o)
                    ent = opairs.get(key)
                    if ent is None:
                        ent = opairs[key] = [
                            out_pool.tile(
                                [DT, 2, SQ], BF16, name=f"osb_{b}_{c // 2}_{o}",
                                tag="osb",
                            ),
                            0,
                        ]
                    osb = ent[0]
                    if scalar_copy:
                        nc.scalar.activation(
                            osb[:, c % 2, :], ops[:], mybir.ActivationFunctionType.Copy
                        )
                    else:
                        nc.vector.tensor_copy(osb[:, c % 2, :], ops[:])
                    ent[1] += 1
                    if ent[1] == 2:
                        del opairs[key]
                        q = nc.sync if o % 2 == 0 else nc.gpsimd
                        q.dma_start(
                            outp[
                                o * DT:(o + 1) * DT,
                                b * S + (c // 2) * 2 * SQ: b * S + (c // 2 + 1) * 2 * SQ,
                            ],
                            osb[:],
                        )
                return go

            return [
                oproj_piece(
                    o,
                    scalar_copy=(last and o % 2 == 1),
                    use_sc_psum=(last and o % 2 == 1),
                )
                for o in range(D // DT)
            ]

        # software pipeline: the global queue runs two qkv chunks ahead of
        # the attention units, plus deferred output projections.  The last
        # batch's units are rotated so the final unit is a small one (J=4)
        # and the kernel tail stays dense.
        NU = B * NQ
        unit_order = [(b, c) for b in range(B) for c in range(NQ)]
        unit_order = unit_order[:-NQ] + unit_order[-NQ + 1:] + [unit_order[-NQ]]
        chunk_order = [(b, c) for b in range(B) for c in range(NQ)]

        for p in qkv_chunk_pieces(0, 0):
            p()
        nc.scalar.dma_start(wo_sb[:], wo[:])
        fill_q += [((0, 1), p) for p in qkv_chunk_pieces(0, 1)]
        nc.gpsimd.dma_start(masks_sb[:, 1:NQ, :, :], masks[:, 1:NQ, :, :])

        for i, (b, c) in enumerate(unit_order):
            if i + 2 < NU:
                ch = chunk_order[i + 2]
                fill_q += [(ch, p) for p in qkv_chunk_pieces(*ch)]
            # guard: every chunk this unit reads must be emitted before
            # the unit's first score matmul
            while any(
                t is not None and t[0] == b and t[1] <= c for t, _ in fill_q
            ):
                pop_one()
            tail = emit_attn_unit(
                b, c, reserve=8 if i < NU - 2 else 0, last=(i == NU - 1)
            )
            fill_q += [(None, p) for p in tail]
        while fill_q:
            pop_one()

    nc.finalize()
    return nc


def _host_inputs(x, Wq, Wk, Wv, Wo):
    import ml_dtypes

    bf = ml_dtypes.bfloat16
    # [chunk, p, t, n]: per-chunk contiguous tiles of x^T
    xT = np.ascontiguousarray(
        x.reshape(B * NQ, SQ, ND, DT).transpose(0, 3, 2, 1)
    ).astype(bf)
    p = np.arange(KT)[:, None, None]
    rr = np.arange(NQ)[None, :, None]
    cc = np.arange(SQ)[None, None, :]
    masks = (cc >= KT * rr + p).astype(bf)
    # duplicated per head so the mask multiply is one [KT, 2, n] DVE op
    masks = np.ascontiguousarray(np.repeat(masks[:, :, None, :], 2, axis=2))
    in_maps = []
    for core in range(NCORES):
        hs = slice(core * HL, (core + 1) * HL)
        wq = Wq[hs].reshape(EL, D).T
        wk = Wk[hs].reshape(EL, D).T
        wv = Wv[hs].reshape(EL, D).T
        wqkv = np.ascontiguousarray(
            np.concatenate([wq, wk, wv], axis=1)
            .reshape(ND, DT, 3 * EL)
            .transpose(1, 0, 2)
        ).astype(bf)
        woL = np.ascontiguousarray(
            Wo[:, core * EL:(core + 1) * EL].T
        ).astype(bf)
        in_maps.append({"xT": xT, "wqkv": wqkv, "wo": woL, "masks": masks})
    return in_maps


def kernel(x, Wq, Wk, Wv, Wo):
    global LAST_RESULTS
    x, Wq, Wk, Wv, Wo = (np.asarray(a, dtype=np.float32) for a in (x, Wq, Wk, Wv, Wo))
    nc = build()
    in_maps = _host_inputs(x, Wq, Wk, Wv, Wo)
    import os
    res = run_bass_kernel_spmd(
        nc, in_maps, list(range(NCORES)),
        trace=bool(os.environ.get("BASS_KERNEL_TRACE")),
    )
    LAST_RESULTS = res
    acc = np.zeros((D, B * S), np.float32)
    for rmap in res.results:
        acc += rmap["outp"]
    return np.ascontiguousarray(acc.T).reshape(B, S, D)


if __name__ == "__main__":
    rng = np.random.default_rng(0)
    scale = 1.0 / np.sqrt(D)
    x = rng.standard_normal((B, S, D), dtype=np.float32)
    Wq = rng.standard_normal((H, E, D), dtype=np.float32) * scale
    Wk = rng.standard_normal((H, E, D), dtype=np.float32) * scale
    Wv = rng.standard_normal((H, E, D), dtype=np.float32) * scale
    Wo = rng.standard_normal((D, D), dtype=np.float32) * scale
    out = kernel(x, Wq, Wk, Wv, Wo)
    print(out.shape, out.dtype, float(np.abs(out).max()))



# revision 39
# speedup vs baseline: 1.3459x; 1.0027x over previous
"""Multi-head causal attention (B=4, S=2048, D=1024, H=16) on 8 TRN2 cores.

Sharding: tensor-parallel over heads (2 heads/core), proj_out row-parallel
with the cross-core reduction done host-side during unsharding.

Per-core kernel layout (all contractions on the SBUF partition axis):
  xT      (1024 d, 8192 tok)   host-pretransposed activations (shared input)
  qT/kT   (128 e2, 2048 s)     per batch; e2 = 2 heads x 64
  scoresT (128 sk, 512 sq)     kv-major scores -> exp -> PV matmul directly
  denom   ones-matmul broadcast of the per-column sums of exp(scores)
  ctxT    (128 e2, 512 sq)     normalized, fed straight into row-parallel Wo
  outp    (1024 o, 8192 tok)   per-core partial; host sums over cores

Head pairs run concurrently on the PE via automatic tile_position (row
tiles for the 64-contraction score matmuls, col tiles for the 64-wide
den/PV matmuls).  Pipeline: warmup matmuls keep the HAM clock-gate warm
through the initial DMA; qkv projection + deferred output-projection
work is interleaved at fine grain between attention iterations so the
PE never idles long enough to re-throttle.

v2: den/PV lag scores by 2 iterations (exp+mask off the PE critical
path), a filler precedes each score pair (LDWEIGHTS cover), output
DMAs move 2-chunk 2KB rows split across sync+gpsimd queues, warmup
deps issue first on the vector queue.
"""

import sys

if "/opt/trn_rl_repo" not in sys.path:
    sys.path.insert(0, "/opt/trn_rl_repo")

from contextlib import ExitStack

import numpy as np

import concourse.bass as bass
import concourse.bacc as bacc
import concourse.mybir as mybir
import concourse.tile as tile
from concourse.bass_utils import run_bass_kernel_spmd
from concourse.masks import make_identity

B, S, D, H, E = 4, 2048, 1024, 16, 64
NCORES = 8
HL = H // NCORES          # heads per core = 2
EL = HL * E               # local feature width = 128
SQ = 512                  # query chunk (matmul moving dim)
NQ = S // SQ              # 4
KT = 128                  # kv tile (contraction tile)
DT = 128                  # d-model contraction tile
ND = D // DT              # 8
N_WARM = 25               # HAM warmup matmuls during initial DMA
F32 = mybir.dt.float32
BF16 = mybir.dt.bfloat16
EXP = mybir.ActivationFunctionType.Exp

LAST_RESULTS = None


def build():
    nc = bacc.Bacc()
    xT = nc.declare_dram_parameter("xT", [B * NQ, DT, ND, SQ], BF16, isOutput=False)
    wqkv = nc.declare_dram_parameter("wqkv", [DT, ND, 3 * EL], BF16, isOutput=False)
    wo = nc.declare_dram_parameter("wo", [EL, D], BF16, isOutput=False)
    masks = nc.declare_dram_parameter("masks", [KT, NQ, 2, SQ], BF16, isOutput=False)
    outp = nc.declare_dram_parameter("outp", [D, B * S], BF16, isOutput=True)

    with tile.TileContext(nc) as tc, ExitStack() as ctx:
        consts = ctx.enter_context(tc.tile_pool(name="consts", bufs=1))
        xt_pool = ctx.enter_context(tc.tile_pool(name="xt", bufs=3))
        qk_pool = ctx.enter_context(tc.tile_pool(name="qk", bufs=2))
        ex_pool = ctx.enter_context(tc.tile_pool(name="ex", bufs=8))
        misc_pool = ctx.enter_context(tc.tile_pool(name="misc", bufs=2))
        out_pool = ctx.enter_context(tc.tile_pool(name="outsb", bufs=16))
        mm_psum = ctx.enter_context(tc.tile_pool(name="mmps", bufs=2, space="PSUM"))
        sc_psum = ctx.enter_context(tc.tile_pool(name="scps", bufs=2, space="PSUM"))
        acc_psum = ctx.enter_context(tc.tile_pool(name="accps", bufs=1, space="PSUM"))
        den_psum = ctx.enter_context(tc.tile_pool(name="denps", bufs=1, space="PSUM"))

        # DMA queue plan for the startup window: three queues in parallel
        # (sync/gpsimd carry the first x chunk halves, the scalar HWDGE
        # ring carries wqkv + the first mask row + wo) so the first
        # projection matmuls are gated on ~0.5MB per queue instead of a
        # serial 1.25MB.
        wqkv_sb = consts.tile([DT, ND, 3 * EL], BF16)
        nc.scalar.dma_start(wqkv_sb[:, 0:ND // 2, :], wqkv[:, 0:ND // 2, :])
        masks_sb = consts.tile([KT, NQ, 2, SQ], BF16)
        wo_sb = consts.tile([EL, D], BF16)
        ones_sb = consts.tile([DT, DT], BF16)
        nc.vector.memset(ones_sb[:], 1.0)

        qkv_tiles = {}
        xt_tiles = {}

        def load_chunk(b, c):
            xt8 = xt_pool.tile([DT, ND, SQ], BF16, name=f"xt_{b}_{c}", tag="xt")
            # x is host-prechunked to [chunk, p, t, n] so each load is one
            # contiguous 8KB-per-partition transfer; split across HWDGE
            # (sync) and SWDGE (gpsimd) so the two halves stream in
            # parallel and the t<4 matmuls start after half has landed
            half = ND // 2
            u = b * NQ + c
            nc.sync.dma_start(xt8[:, 0:half, :], xT[u, :, 0:half, :])
            nc.gpsimd.dma_start(xt8[:, half:ND, :], xT[u, :, half:ND, :])
            xt_tiles[(b, c)] = xt8
            return xt8

        load_chunk(0, 0)
        nc.scalar.dma_start(wqkv_sb[:, ND // 2:ND, :], wqkv[:, ND // 2:ND, :])
        nc.scalar.dma_start(masks_sb[:, 0:1, :, :], masks[:, 0:1, :, :])

        # HAM warmup: keep the PE busy from the earliest possible moment
        # (gated only on the ones memset) until the first x chunk lands
        # (~17us) so the clock-gate is at 8/8 when real work issues.
        warm_src = consts.tile([DT, SQ], BF16)
        nc.vector.memset(warm_src[:], 0.0)
        warm_ps = mm_psum.tile([DT, SQ], F32, name="warm_ps", tag="mm")
        for _ in range(N_WARM):
            nc.tensor.matmul(warm_ps[:], ones_sb[:], warm_src[:], start=True, stop=True)

        ident = consts.tile([DT, DT], BF16)
        make_identity(nc, ident[:])

        def qkv_chunk_pieces(b, c):
            # returns fine-grained filler closures; each emits a small piece
            # of the qkv work for chunk (b, c) so it can be sprinkled between
            # attention iterations (engine FIFOs are strict in-order)
            if c == 0:
                qT = qk_pool.tile([EL, S], BF16, name=f"qT_{b}", tag="qT")
                kT = qk_pool.tile([EL, S], BF16, name=f"kT_{b}", tag="kT")
                vT = qk_pool.tile([EL, S], BF16, name=f"vT_{b}", tag="vT", bufs=1)
                v_sb = qk_pool.tile([KT, S // KT, EL], BF16, name=f"v_{b}", tag="v")
                qkv_tiles[b] = (qT, kT, vT, v_sb)
            qT, kT, vT, v_sb = qkv_tiles[b]
            xt8 = xt_tiles.get((b, c))
            if xt8 is None:
                xt8 = load_chunk(b, c)

            psums = {}

            def proj_piece(dest, col0, t0, t1):
                # emitted as col-tile pairs (two concurrent [128,64] tiles)
                # so fillers share the den/PV tile config -- avoids the
                # ~110ns PE array-reconfigure penalty per switch
                def go():
                    if t0 == 0:
                        psums[col0] = mm_psum.tile(
                            [EL, SQ], F32, name=f"qkv_ps_{b}_{c}_{col0}", tag="mm"
                        )
                    ps = psums[col0]
                    for t in range(t0, t1):
                        for h in range(2):
                            nc.tensor.matmul(
                                ps[h * E:(h + 1) * E, :],
                                wqkv_sb[:, t, col0 + h * E:col0 + (h + 1) * E],
                                xt8[:, t, :],
                                start=(t == 0),
                                stop=(t == ND - 1),
                                skip_group_check=True,
                            )
                    if t1 == ND:
                        nc.vector.tensor_copy(dest[:, c * SQ:(c + 1) * SQ], ps[:])
                return go

            def vtr4():
                # all four transposes in one piece: transpose-mode is its
                # own PE tile config, so batching them pays the reconfigure
                # penalty once per chunk instead of per pair
                def go():
                    for j in range(4 * c, 4 * c + 4):
                        vt_ps = mm_psum.tile([KT, KT], BF16, name=f"vt_ps_{b}_{j}", tag="mm")
                        nc.tensor.transpose(vt_ps[:], vT[:, j * KT:(j + 1) * KT], ident[:])
                        nc.vector.tensor_copy(v_sb[:, j, :], vt_ps[:])
                return go

            pieces = []
            for col0, dest in ((0, qT), (EL, kT), (2 * EL, vT)):
                for t0 in range(0, ND, 4):
                    pieces.append(proj_piece(dest, col0, t0, t0 + 4))
            pieces.append(vtr4())
            return pieces

        # global filler queue: (chunk_tag_or_None, closure).  Chunk pieces
        # and deferred output projections pop between attention iterations
        # at a self-balancing cadence; `reserve` pieces are held back so
        # later units never starve.
        fill_q = []
        # output staging: adjacent q-chunk pairs share one [DT, 2*SQ] tile
        # so each outp DMA moves 2KB rows (half the descriptor count);
        # pairs alternate between the sync and gpsimd queues.
        opairs = {}

        def pop_one():
            if fill_q:
                fill_q.pop(0)[1]()

        def emit_attn_unit(b, c, reserve=0, last=False):
            # returns tail closures (the row-parallel output projection) to
            # be deferred into later units' iteration loops
            qT, kT, vT, v_sb = qkv_tiles[b]
            J = (c + 1) * (SQ // KT)  # causal kv tiles for this chunk
            ctx_ps = acc_psum.tile([2 * E, SQ], F32, name=f"ctx_{b}_{c}", tag="ctx")
            denb = den_psum.tile([KT, SQ], F32, name=f"den_{b}_{c}", tag="den")
            def emit_denpv(idx, j, ex, cut):
                # denominator rides PE: ones.T @ ex accumulates the
                # per-column sums, already broadcast over partitions.
                # start/stop key on EMISSION order (idx), not kv index.
                for h in range(HL):
                    nc.tensor.matmul(
                        denb[h * E:(h + 1) * E, cut:SQ],
                        ones_sb[:, h * E:(h + 1) * E],
                        ex[:, h, cut:SQ],
                        start=(idx == 0),
                        stop=(idx == J - 1),
                        skip_group_check=True,
                    )
                for h in range(HL):
                    nc.tensor.matmul(
                        ctx_ps[h * E:(h + 1) * E, cut:SQ],
                        v_sb[:, j, h * E:(h + 1) * E],
                        ex[:, h, cut:SQ],
                        start=(idx == 0),
                        stop=(idx == J - 1),
                        skip_group_check=True,
                    )

            # diagonal kv tiles (small, exp/mask-paced) run FIRST while the
            # filler queue is full; the dense full-width tiles close the
            # unit back-to-back once fillers thin out
            js = list(range(max(0, J - 4), J)) + list(range(0, max(0, J - 4)))
            # kv tiles are processed in MACRO pairs with den/PV lagging by
            # one macro: the PE sees [sc,sc][den,pv,den,pv][fillers] with
            # only two tile-config switches (row<->col) per macro, the
            # second block of each kind paying no reconfigure penalty, and
            # exp (scalar) + mask (vector) a full macro off the critical
            # path.
            pending = []

            def emit_sc(idx, j):
                rdiag = j - (c * (SQ // KT))
                # columns [0, cut) of this q-chunk are fully masked for
                # diagonal kv tiles -- skip them everywhere
                cut = KT * rdiag if rdiag > 0 else 0
                n = SQ - cut
                sc = sc_psum.tile([KT, 2, SQ], F32, name=f"sc_{b}_{c}_{j}", tag="sc")
                ex = ex_pool.tile([KT, 2, SQ], BF16, name=f"ex_{b}_{c}_{j}", tag="ex")
                for h in range(HL):
                    nc.tensor.matmul(
                        sc[:, h, 0:n],
                        kT[h * E:(h + 1) * E, j * KT:(j + 1) * KT],
                        qT[h * E:(h + 1) * E, c * SQ + cut:(c + 1) * SQ],
                        start=True,
                        stop=True,
                    )
                nc.scalar.activation(
                    ex[:, :, cut:SQ], sc[:, :, 0:n], EXP, scale=0.125
                )
                if rdiag >= 0:
                    nc.vector.tensor_mul(
                        ex[:, :, cut:SQ],
                        ex[:, :, cut:SQ],
                        masks_sb[:, rdiag, :, cut:SQ],
                    )
                pending.append((idx, j, ex, cut))

            for idx, j in enumerate(js):
                pop_one()
                emit_sc(idx, j)
                # on the unit's last iteration, drain the den/PV pipeline
                # right away (with filler cover for the fresh exps) so the
                # recb/ctx chain starts ~2 iterations earlier and the next
                # unit's den never waits on the acc/den psum banks
                lag = 0 if idx == J - 1 else 2
                if lag == 0:
                    pop_one()
                    pop_one()
                while len(pending) > lag:
                    emit_denpv(*pending.pop(0))
                # self-balancing filler cadence: spread the queue (minus
                # the held-back reserve) evenly over remaining iterations
                quota = -(-max(0, len(fill_q) - reserve) // (J - idx)) - 1
                for _ in range(quota):
                    pop_one()
            for p in pending:
                emit_denpv(*p)

            recb = misc_pool.tile([KT, SQ], F32, name=f"rec_{b}_{c}", tag="recb")
            nc.vector.reciprocal_approx_fast(recb[:], denb[:])
            ctx_sb = misc_pool.tile(
                [2 * E, SQ], BF16, name=f"ctxsb_{b}_{c}", tag="ctxsb", bufs=3
            )
            nc.vector.tensor_mul(ctx_sb[:], ctx_ps[:], recb[:])

            # ---- row-parallel output projection (partial), deferred ----
            # These pieces pop during later units.  When they land in an
            # ACT-idle phase (after a c==3 unit, or the final flush), the
            # PSUM->SBUF copy goes to the scalar engine so the vector
            # engine's copy backlog doesn't stall the mm_psum rotation;
            # the final flush also borrows the (now idle) score banks.
            def oproj_piece(o, scalar_copy=False, use_sc_psum=False):
                def go():
                    pool = sc_psum if use_sc_psum else mm_psum
                    tag = "sc" if use_sc_psum else "mm"
                    ops = pool.tile([DT, SQ], F32, name=f"op_{b}_{c}_{o}", tag=tag)
                    for h in range(2):
                        nc.tensor.matmul(
                            ops[h * E:(h + 1) * E, :],
                            wo_sb[:, o * DT + h * E:o * DT + (h + 1) * E],
                            ctx_sb[:],
                            start=True, stop=True,
                            skip_group_check=True,
                        )
                    # the rotated last unit (B-1, 0) runs ~12 units after
                    # its pair partner (B-1, 1): fire those halves as
                    # singles so the partner's data doesn't sit in SBUF
                    # until the kernel tail
                    if b == B - 1 and c < 2:
                        osb = out_pool.tile(
                            [DT, SQ], BF16, name=f"osbS_{b}_{c}_{o}", tag="osb"
                        )
                        if scalar_copy:
                            nc.scalar.activation(
                                osb[:], ops[:], mybir.ActivationFunctionType.Copy
                            )
                        else:
                            nc.vector.tensor_copy(osb[:], ops[:])
                        if last:
                            # kernel tail: exp work is done, so the scalar
                            # queue is free to help drain the final outputs
                            q = (nc.sync, nc.gpsimd, nc.scalar)[o % 3]
                        else:
                            q = nc.sync if o % 2 == 0 else nc.gpsimd
                        q.dma_start(
                            outp[
                                o * DT:(o + 1) * DT,
                                b * S + c * SQ: b * S + (c + 1) * SQ,
                            ],
                            osb[:],
                        )
                        return
                    key = (b, c // 2, o)
                    ent = opairs.get(key)
                    if ent is None:
                        ent = opairs[key] = [
                            out_pool.tile(
                                [DT, 2, SQ], BF16, name=f"osb_{b}_{c // 2}_{o}",
                                tag="osb",
                            ),
                            0,
                        ]
                    osb = ent[0]
                    if scalar_copy:
                        nc.scalar.activation(
                            osb[:, c % 2, :], ops[:], mybir.ActivationFunctionType.Copy
                        )
                    else:
                        nc.vector.tensor_copy(osb[:, c % 2, :], ops[:])
                    ent[1] += 1
                    if ent[1] == 2:
                        del opairs[key]
                        q = nc.sync if o % 2 == 0 else nc.gpsimd
                        q.dma_start(
                            outp[
                                o * DT:(o + 1) * DT,
                                b * S + (c // 2) * 2 * SQ: b * S + (c // 2 + 1) * 2 * SQ,
                            ],
                            osb[:],
                        )
                return go

            return [
                oproj_piece(
                    o,
                    scalar_copy=(last and o % 2 == 1),
                    use_sc_psum=(last and o % 2 == 1),
                )
                for o in range(D // DT)
            ]

        # software pipeline: the global queue runs two qkv chunks ahead of
        # the attention units, plus deferred output projections.  The last
        # batch's units are rotated so the final unit is a small one (J=4)
        # and the kernel tail stays dense.
        NU = B * NQ
        unit_order = [(b, c) for b in range(B) for c in range(NQ)]
        unit_order = unit_order[:-NQ] + unit_order[-NQ + 1:] + [unit_order[-NQ]]
        chunk_order = [(b, c) for b in range(B) for c in range(NQ)]

        for p in qkv_chunk_pieces(0, 0):
            p()
        nc.scalar.dma_start(wo_sb[:], wo[:])
        fill_q += [((0, 1), p) for p in qkv_chunk_pieces(0, 1)]
        nc.gpsimd.dma_start(masks_sb[:, 1:NQ, :, :], masks[:, 1:NQ, :, :])

        for i, (b, c) in enumerate(unit_order):
            if i + 2 < NU:
                ch = chunk_order[i + 2]
                fill_q += [(ch, p) for p in qkv_chunk_pieces(*ch)]
            # guard: every chunk this unit reads must be emitted before
            # the unit's first score matmul
            while any(
                t is not None and t[0] == b and t[1] <= c for t, _ in fill_q
            ):
                pop_one()
            # the last two units keep a few fillers in reserve so the PE
            # has cover while their recb/ctx chains drain on the DVE
            tail = emit_attn_unit(
                b, c, reserve=(8, 4, 3)[max(0, i - (NU - 3))], last=(i == NU - 1)
            )
            fill_q += [(None, p) for p in tail]
        while fill_q:
            pop_one()

    nc.finalize()
    return nc


def _host_inputs(x, Wq, Wk, Wv, Wo):
    import ml_dtypes

    bf = ml_dtypes.bfloat16
    # [chunk, p, t, n]: per-chunk contiguous tiles of x^T
    xT = np.ascontiguousarray(
        x.reshape(B * NQ, SQ, ND, DT).transpose(0, 3, 2, 1)
    ).astype(bf)
    p = np.arange(KT)[:, None, None]
    rr = np.arange(NQ)[None, :, None]
    cc = np.arange(SQ)[None, None, :]
    masks = (cc >= KT * rr + p).astype(bf)
    # duplicated per head so the mask multiply is one [KT, 2, n] DVE op
    masks = np.ascontiguousarray(np.repeat(masks[:, :, None, :], 2, axis=2))
    in_maps = []
    for core in range(NCORES):
        hs = slice(core * HL, (core + 1) * HL)
        wq = Wq[hs].reshape(EL, D).T
        wk = Wk[hs].reshape(EL, D).T
        wv = Wv[hs].reshape(EL, D).T
        wqkv = np.ascontiguousarray(
            np.concatenate([wq, wk, wv], axis=1)
            .reshape(ND, DT, 3 * EL)
            .transpose(1, 0, 2)
        ).astype(bf)
        woL = np.ascontiguousarray(
            Wo[:, core * EL:(core + 1) * EL].T
        ).astype(bf)
        in_maps.append({"xT": xT, "wqkv": wqkv, "wo": woL, "masks": masks})
    return in_maps


def kernel(x, Wq, Wk, Wv, Wo):
    global LAST_RESULTS
    x, Wq, Wk, Wv, Wo = (np.asarray(a, dtype=np.float32) for a in (x, Wq, Wk, Wv, Wo))
    nc = build()
    in_maps = _host_inputs(x, Wq, Wk, Wv, Wo)
    import os
    res = run_bass_kernel_spmd(
        nc, in_maps, list(range(NCORES)),
        trace=bool(os.environ.get("BASS_KERNEL_TRACE")),
    )
    LAST_RESULTS = res
    acc = np.zeros((D, B * S), np.float32)
    for rmap in res.results:
        acc += rmap["outp"]
    return np.ascontiguousarray(acc.T).reshape(B, S, D)


if __name__ == "__main__":
    rng = np.random.default_rng(0)
    scale = 1.0 / np.sqrt(D)
    x = rng.standard_normal((B, S, D), dtype=np.float32)
    Wq = rng.standard_normal((H, E, D), dtype=np.float32) * scale
    Wk = rng.standard_normal((H, E, D), dtype=np.float32) * scale
    Wv = rng.standard_normal((H, E, D), dtype=np.float32) * scale
    Wo = rng.standard_normal((D, D), dtype=np.float32) * scale
    out = kernel(x, Wq, Wk, Wv, Wo)
    print(out.shape, out.dtype, float(np.abs(out).max()))



# revision 41
# speedup vs baseline: 1.3498x; 1.0029x over previous
"""Multi-head causal attention (B=4, S=2048, D=1024, H=16) on 8 TRN2 cores.

Sharding: tensor-parallel over heads (2 heads/core), proj_out row-parallel
with the cross-core reduction done host-side during unsharding.

Per-core kernel layout (all contractions on the SBUF partition axis):
  xT      (1024 d, 8192 tok)   host-pretransposed activations (shared input)
  qT/kT   (128 e2, 2048 s)     per batch; e2 = 2 heads x 64
  scoresT (128 sk, 512 sq)     kv-major scores -> exp -> PV matmul directly
  denom   ones-matmul broadcast of the per-column sums of exp(scores)
  ctxT    (128 e2, 512 sq)     normalized, fed straight into row-parallel Wo
  outp    (1024 o, 8192 tok)   per-core partial; host sums over cores

Head pairs run concurrently on the PE via automatic tile_position (row
tiles for the 64-contraction score matmuls, col tiles for the 64-wide
den/PV matmuls).  Pipeline: warmup matmuls keep the HAM clock-gate warm
through the initial DMA; qkv projection + deferred output-projection
work is interleaved at fine grain between attention iterations so the
PE never idles long enough to re-throttle.

v2: den/PV lag scores by 2 iterations (exp+mask off the PE critical
path), a filler precedes each score pair (LDWEIGHTS cover), output
DMAs move 2-chunk 2KB rows split across sync+gpsimd queues, warmup
deps issue first on the vector queue.
"""

import sys

if "/opt/trn_rl_repo" not in sys.path:
    sys.path.insert(0, "/opt/trn_rl_repo")

from contextlib import ExitStack

import numpy as np

import concourse.bass as bass
import concourse.bacc as bacc
import concourse.mybir as mybir
import concourse.tile as tile
from concourse.bass_utils import run_bass_kernel_spmd
from concourse.masks import make_identity

B, S, D, H, E = 4, 2048, 1024, 16, 64
NCORES = 8
HL = H // NCORES          # heads per core = 2
EL = HL * E               # local feature width = 128
SQ = 512                  # query chunk (matmul moving dim)
NQ = S // SQ              # 4
KT = 128                  # kv tile (contraction tile)
DT = 128                  # d-model contraction tile
ND = D // DT              # 8
N_WARM = 33               # HAM warmup matmuls during initial DMA
F32 = mybir.dt.float32
BF16 = mybir.dt.bfloat16
EXP = mybir.ActivationFunctionType.Exp

LAST_RESULTS = None


def build():
    nc = bacc.Bacc()
    xT = nc.declare_dram_parameter("xT", [B * NQ, DT, ND, SQ], BF16, isOutput=False)
    wqkv = nc.declare_dram_parameter("wqkv", [DT, ND, 3 * EL], BF16, isOutput=False)
    wo = nc.declare_dram_parameter("wo", [EL, D], BF16, isOutput=False)
    masks = nc.declare_dram_parameter("masks", [KT, NQ, 2, SQ], BF16, isOutput=False)
    outp = nc.declare_dram_parameter("outp", [D, B * S], BF16, isOutput=True)

    with tile.TileContext(nc) as tc, ExitStack() as ctx:
        consts = ctx.enter_context(tc.tile_pool(name="consts", bufs=1))
        xt_pool = ctx.enter_context(tc.tile_pool(name="xt", bufs=3))
        qk_pool = ctx.enter_context(tc.tile_pool(name="qk", bufs=2))
        ex_pool = ctx.enter_context(tc.tile_pool(name="ex", bufs=8))
        misc_pool = ctx.enter_context(tc.tile_pool(name="misc", bufs=2))
        out_pool = ctx.enter_context(tc.tile_pool(name="outsb", bufs=16))
        mm_psum = ctx.enter_context(tc.tile_pool(name="mmps", bufs=2, space="PSUM"))
        sc_psum = ctx.enter_context(tc.tile_pool(name="scps", bufs=2, space="PSUM"))
        acc_psum = ctx.enter_context(tc.tile_pool(name="accps", bufs=1, space="PSUM"))
        den_psum = ctx.enter_context(tc.tile_pool(name="denps", bufs=1, space="PSUM"))

        # DMA queue plan for the startup window: three queues in parallel
        # (sync/gpsimd carry the first x chunk halves, the scalar HWDGE
        # ring carries wqkv + the first mask row + wo) so the first
        # projection matmuls are gated on ~0.5MB per queue instead of a
        # serial 1.25MB.
        wqkv_sb = consts.tile([DT, ND, 3 * EL], BF16)
        nc.scalar.dma_start(wqkv_sb[:, 0:ND // 2, :], wqkv[:, 0:ND // 2, :])
        masks_sb = consts.tile([KT, NQ, 2, SQ], BF16)
        wo_sb = consts.tile([EL, D], BF16)
        ones_sb = consts.tile([DT, DT], BF16)
        nc.vector.memset(ones_sb[:], 1.0)

        qkv_tiles = {}
        xt_tiles = {}

        def load_chunk(b, c):
            xt8 = xt_pool.tile([DT, ND, SQ], BF16, name=f"xt_{b}_{c}", tag="xt")
            # x is host-prechunked to [chunk, p, t, n] so each load is one
            # contiguous 8KB-per-partition transfer; split across HWDGE
            # (sync) and SWDGE (gpsimd) so the two halves stream in
            # parallel and the t<4 matmuls start after half has landed
            half = ND // 2
            u = b * NQ + c
            nc.sync.dma_start(xt8[:, 0:half, :], xT[u, :, 0:half, :])
            nc.gpsimd.dma_start(xt8[:, half:ND, :], xT[u, :, half:ND, :])
            xt_tiles[(b, c)] = xt8
            return xt8

        load_chunk(0, 0)
        nc.scalar.dma_start(wqkv_sb[:, ND // 2:ND, :], wqkv[:, ND // 2:ND, :])
        nc.scalar.dma_start(masks_sb[:, 0:1, :, :], masks[:, 0:1, :, :])

        # HAM warmup: keep the PE busy from the earliest possible moment
        # (gated only on the ones memset) until the first x chunk lands
        # (~17us) so the clock-gate is at 8/8 when real work issues.
        warm_src = consts.tile([DT, SQ], BF16)
        nc.vector.memset(warm_src[:], 0.0)
        warm_ps = mm_psum.tile([DT, SQ], F32, name="warm_ps", tag="mm")
        for _ in range(N_WARM):
            nc.tensor.matmul(warm_ps[:], ones_sb[:], warm_src[:], start=True, stop=True)

        ident = consts.tile([DT, DT], BF16)
        make_identity(nc, ident[:])

        def qkv_chunk_pieces(b, c):
            # returns fine-grained filler closures; each emits a small piece
            # of the qkv work for chunk (b, c) so it can be sprinkled between
            # attention iterations (engine FIFOs are strict in-order)
            if c == 0:
                qT = qk_pool.tile([EL, S], BF16, name=f"qT_{b}", tag="qT")
                kT = qk_pool.tile([EL, S], BF16, name=f"kT_{b}", tag="kT")
                vT = qk_pool.tile([EL, S], BF16, name=f"vT_{b}", tag="vT", bufs=1)
                v_sb = qk_pool.tile([KT, S // KT, EL], BF16, name=f"v_{b}", tag="v")
                qkv_tiles[b] = (qT, kT, vT, v_sb)
            qT, kT, vT, v_sb = qkv_tiles[b]
            xt8 = xt_tiles.get((b, c))
            if xt8 is None:
                xt8 = load_chunk(b, c)

            psums = {}

            def proj_piece(dest, col0, t0, t1):
                # emitted as col-tile pairs (two concurrent [128,64] tiles)
                # so fillers share the den/PV tile config -- avoids the
                # ~110ns PE array-reconfigure penalty per switch
                def go():
                    if t0 == 0:
                        psums[col0] = mm_psum.tile(
                            [EL, SQ], F32, name=f"qkv_ps_{b}_{c}_{col0}", tag="mm"
                        )
                    ps = psums[col0]
                    for t in range(t0, t1):
                        for h in range(2):
                            nc.tensor.matmul(
                                ps[h * E:(h + 1) * E, :],
                                wqkv_sb[:, t, col0 + h * E:col0 + (h + 1) * E],
                                xt8[:, t, :],
                                start=(t == 0),
                                stop=(t == ND - 1),
                                skip_group_check=True,
                            )
                    if t1 == ND:
                        nc.vector.tensor_copy(dest[:, c * SQ:(c + 1) * SQ], ps[:])
                return go

            def vtr4():
                # all four transposes in one piece: transpose-mode is its
                # own PE tile config, so batching them pays the reconfigure
                # penalty once per chunk instead of per pair
                def go():
                    for j in range(4 * c, 4 * c + 4):
                        vt_ps = mm_psum.tile([KT, KT], BF16, name=f"vt_ps_{b}_{j}", tag="mm")
                        nc.tensor.transpose(vt_ps[:], vT[:, j * KT:(j + 1) * KT], ident[:])
                        nc.vector.tensor_copy(v_sb[:, j, :], vt_ps[:])
                return go

            pieces = []
            for col0, dest in ((0, qT), (EL, kT), (2 * EL, vT)):
                for t0 in range(0, ND, 4):
                    pieces.append(proj_piece(dest, col0, t0, t0 + 4))
            pieces.append(vtr4())
            return pieces

        # global filler queue: (chunk_tag_or_None, closure).  Chunk pieces
        # and deferred output projections pop between attention iterations
        # at a self-balancing cadence; `reserve` pieces are held back so
        # later units never starve.
        fill_q = []
        # output staging: adjacent q-chunk pairs share one [DT, 2*SQ] tile
        # so each outp DMA moves 2KB rows (half the descriptor count);
        # pairs alternate between the sync and gpsimd queues.
        opairs = {}

        def pop_one():
            if fill_q:
                fill_q.pop(0)[1]()

        def emit_attn_unit(b, c, reserve=0, last=False):
            # returns tail closures (the row-parallel output projection) to
            # be deferred into later units' iteration loops
            qT, kT, vT, v_sb = qkv_tiles[b]
            J = (c + 1) * (SQ // KT)  # causal kv tiles for this chunk
            ctx_ps = acc_psum.tile([2 * E, SQ], F32, name=f"ctx_{b}_{c}", tag="ctx")
            denb = den_psum.tile([KT, SQ], F32, name=f"den_{b}_{c}", tag="den")
            def emit_denpv(idx, j, ex, cut):
                # denominator rides PE: ones.T @ ex accumulates the
                # per-column sums, already broadcast over partitions.
                # start/stop key on EMISSION order (idx), not kv index.
                for h in range(HL):
                    nc.tensor.matmul(
                        denb[h * E:(h + 1) * E, cut:SQ],
                        ones_sb[:, h * E:(h + 1) * E],
                        ex[:, h, cut:SQ],
                        start=(idx == 0),
                        stop=(idx == J - 1),
                        skip_group_check=True,
                    )
                for h in range(HL):
                    nc.tensor.matmul(
                        ctx_ps[h * E:(h + 1) * E, cut:SQ],
                        v_sb[:, j, h * E:(h + 1) * E],
                        ex[:, h, cut:SQ],
                        start=(idx == 0),
                        stop=(idx == J - 1),
                        skip_group_check=True,
                    )

            # diagonal kv tiles (small, exp/mask-paced) run FIRST while the
            # filler queue is full; the dense full-width tiles close the
            # unit back-to-back once fillers thin out
            js = list(range(max(0, J - 4), J)) + list(range(0, max(0, J - 4)))
            # kv tiles are processed in MACRO pairs with den/PV lagging by
            # one macro: the PE sees [sc,sc][den,pv,den,pv][fillers] with
            # only two tile-config switches (row<->col) per macro, the
            # second block of each kind paying no reconfigure penalty, and
            # exp (scalar) + mask (vector) a full macro off the critical
            # path.
            pending = []

            def emit_sc(idx, j):
                rdiag = j - (c * (SQ // KT))
                # columns [0, cut) of this q-chunk are fully masked for
                # diagonal kv tiles -- skip them everywhere
                cut = KT * rdiag if rdiag > 0 else 0
                n = SQ - cut
                sc = sc_psum.tile([KT, 2, SQ], F32, name=f"sc_{b}_{c}_{j}", tag="sc")
                ex = ex_pool.tile([KT, 2, SQ], BF16, name=f"ex_{b}_{c}_{j}", tag="ex")
                for h in range(HL):
                    nc.tensor.matmul(
                        sc[:, h, 0:n],
                        kT[h * E:(h + 1) * E, j * KT:(j + 1) * KT],
                        qT[h * E:(h + 1) * E, c * SQ + cut:(c + 1) * SQ],
                        start=True,
                        stop=True,
                    )
                nc.scalar.activation(
                    ex[:, :, cut:SQ], sc[:, :, 0:n], EXP, scale=0.125
                )
                if rdiag >= 0:
                    nc.vector.tensor_mul(
                        ex[:, :, cut:SQ],
                        ex[:, :, cut:SQ],
                        masks_sb[:, rdiag, :, cut:SQ],
                    )
                pending.append((idx, j, ex, cut))

            # kv tiles run in MACRO pairs: the PE sees [sc,sc][den,pv,
            # den,pv][fillers] with only two tile-config switches
            # (row<->col) per macro -- the second block of each kind pays
            # no reconfigure penalty -- while den/PV still lag their
            # scores by a full macro (exp/mask off the critical path)
            JM = J // 2
            for mi in range(JM):
                pop_one()
                emit_sc(2 * mi, js[2 * mi])
                emit_sc(2 * mi + 1, js[2 * mi + 1])
                # on the unit's last macro, drain the den/PV pipeline
                # right away (with filler cover for the fresh exps) so the
                # recb/ctx chain starts earlier and the next unit's den
                # never waits on the acc/den psum banks
                lag = 0 if mi == JM - 1 else 2
                if lag == 0:
                    pop_one()
                    pop_one()
                while len(pending) > lag:
                    emit_denpv(*pending.pop(0))
                # self-balancing filler cadence: spread the queue (minus
                # the held-back reserve) evenly over remaining macros
                quota = -(-max(0, len(fill_q) - reserve) // (JM - mi)) - 1
                for _ in range(quota):
                    pop_one()
            for p in pending:
                emit_denpv(*p)

            recb = misc_pool.tile([KT, SQ], F32, name=f"rec_{b}_{c}", tag="recb")
            nc.vector.reciprocal_approx_fast(recb[:], denb[:])
            ctx_sb = misc_pool.tile(
                [2 * E, SQ], BF16, name=f"ctxsb_{b}_{c}", tag="ctxsb", bufs=3
            )
            nc.vector.tensor_mul(ctx_sb[:], ctx_ps[:], recb[:])

            # ---- row-parallel output projection (partial), deferred ----
            # These pieces pop during later units.  When they land in an
            # ACT-idle phase (after a c==3 unit, or the final flush), the
            # PSUM->SBUF copy goes to the scalar engine so the vector
            # engine's copy backlog doesn't stall the mm_psum rotation;
            # the final flush also borrows the (now idle) score banks.
            def oproj_piece(o, scalar_copy=False, use_sc_psum=False):
                def go():
                    pool = sc_psum if use_sc_psum else mm_psum
                    tag = "sc" if use_sc_psum else "mm"
                    ops = pool.tile([DT, SQ], F32, name=f"op_{b}_{c}_{o}", tag=tag)
                    for h in range(2):
                        nc.tensor.matmul(
                            ops[h * E:(h + 1) * E, :],
                            wo_sb[:, o * DT + h * E:o * DT + (h + 1) * E],
                            ctx_sb[:],
                            start=True, stop=True,
                            skip_group_check=True,
                        )
                    # the rotated last unit (B-1, 0) runs ~12 units after
                    # its pair partner (B-1, 1): fire those halves as
                    # singles so the partner's data doesn't sit in SBUF
                    # until the kernel tail
                    if b == B - 1 and c < 2:
                        osb = out_pool.tile(
                            [DT, SQ], BF16, name=f"osbS_{b}_{c}_{o}", tag="osb"
                        )
                        if scalar_copy:
                            nc.scalar.activation(
                                osb[:], ops[:], mybir.ActivationFunctionType.Copy
                            )
                        else:
                            nc.vector.tensor_copy(osb[:], ops[:])
                        if last:
                            # kernel tail: exp work is done, so the scalar
                            # queue is free to help drain the final outputs
                            q = (nc.sync, nc.gpsimd, nc.scalar)[o % 3]
                        else:
                            q = nc.sync if o % 2 == 0 else nc.gpsimd
                        q.dma_start(
                            outp[
                                o * DT:(o + 1) * DT,
                                b * S + c * SQ: b * S + (c + 1) * SQ,
                            ],
                            osb[:],
                        )
                        return
                    key = (b, c // 2, o)
                    ent = opairs.get(key)
                    if ent is None:
                        ent = opairs[key] = [
                            out_pool.tile(
                                [DT, 2, SQ], BF16, name=f"osb_{b}_{c // 2}_{o}",
                                tag="osb",
                            ),
                            0,
                        ]
                    osb = ent[0]
                    if scalar_copy:
                        nc.scalar.activation(
                            osb[:, c % 2, :], ops[:], mybir.ActivationFunctionType.Copy
                        )
                    else:
                        nc.vector.tensor_copy(osb[:, c % 2, :], ops[:])
                    ent[1] += 1
                    if ent[1] == 2:
                        del opairs[key]
                        q = nc.sync if o % 2 == 0 else nc.gpsimd
                        q.dma_start(
                            outp[
                                o * DT:(o + 1) * DT,
                                b * S + (c // 2) * 2 * SQ: b * S + (c // 2 + 1) * 2 * SQ,
                            ],
                            osb[:],
                        )
                return go

            return [
                oproj_piece(
                    o,
                    scalar_copy=(last and o % 2 == 1),
                    use_sc_psum=(last and o % 2 == 1),
                )
                for o in range(D // DT)
            ]

        # software pipeline: the global queue runs two qkv chunks ahead of
        # the attention units, plus deferred output projections.  The last
        # batch's units are rotated so the final unit is a small one (J=4)
        # and the kernel tail stays dense.
        NU = B * NQ
        unit_order = [(b, c) for b in range(B) for c in range(NQ)]
        unit_order = unit_order[:-NQ] + unit_order[-NQ + 1:] + [unit_order[-NQ]]
        chunk_order = [(b, c) for b in range(B) for c in range(NQ)]

        for p in qkv_chunk_pieces(0, 0):
            p()
        nc.scalar.dma_start(wo_sb[:], wo[:])
        fill_q += [((0, 1), p) for p in qkv_chunk_pieces(0, 1)]
        nc.gpsimd.dma_start(masks_sb[:, 1:NQ, :, :], masks[:, 1:NQ, :, :])

        for i, (b, c) in enumerate(unit_order):
            if i + 2 < NU:
                ch = chunk_order[i + 2]
                fill_q += [(ch, p) for p in qkv_chunk_pieces(*ch)]
            # guard: every chunk this unit reads must be emitted before
            # the unit's first score matmul
            while any(
                t is not None and t[0] == b and t[1] <= c for t, _ in fill_q
            ):
                pop_one()
            # the last two units keep a few fillers in reserve so the PE
            # has cover while their recb/ctx chains drain on the DVE
            tail = emit_attn_unit(
                b, c, reserve=(8, 4, 3)[max(0, i - (NU - 3))], last=(i == NU - 1)
            )
            fill_q += [(None, p) for p in tail]
        while fill_q:
            pop_one()

    nc.finalize()
    return nc


def _host_inputs(x, Wq, Wk, Wv, Wo):
    import ml_dtypes

    bf = ml_dtypes.bfloat16
    # [chunk, p, t, n]: per-chunk contiguous tiles of x^T
    xT = np.ascontiguousarray(
        x.reshape(B * NQ, SQ, ND, DT).transpose(0, 3, 2, 1)
    ).astype(bf)
    p = np.arange(KT)[:, None, None]
    rr = np.arange(NQ)[None, :, None]
    cc = np.arange(SQ)[None, None, :]
    masks = (cc >= KT * rr + p).astype(bf)
    # duplicated per head so the mask multiply is one [KT, 2, n] DVE op
    masks = np.ascontiguousarray(np.repeat(masks[:, :, None, :], 2, axis=2))
    in_maps = []
    for core in range(NCORES):
        hs = slice(core * HL, (core + 1) * HL)
        wq = Wq[hs].reshape(EL, D).T
        wk = Wk[hs].reshape(EL, D).T
        wv = Wv[hs].reshape(EL, D).T
        wqkv = np.ascontiguousarray(
            np.concatenate([wq, wk, wv], axis=1)
            .reshape(ND, DT, 3 * EL)
            .transpose(1, 0, 2)
        ).astype(bf)
        woL = np.ascontiguousarray(
            Wo[:, core * EL:(core + 1) * EL].T
        ).astype(bf)
        in_maps.append({"xT": xT, "wqkv": wqkv, "wo": woL, "masks": masks})
    return in_maps


def kernel(x, Wq, Wk, Wv, Wo):
    global LAST_RESULTS
    x, Wq, Wk, Wv, Wo = (np.asarray(a, dtype=np.float32) for a in (x, Wq, Wk, Wv, Wo))
    nc = build()
    in_maps = _host_inputs(x, Wq, Wk, Wv, Wo)
    import os
    res = run_bass_kernel_spmd(
        nc, in_maps, list(range(NCORES)),
        trace=bool(os.environ.get("BASS_KERNEL_TRACE")),
    )
    LAST_RESULTS = res
    acc = np.zeros((D, B * S), np.float32)
    for rmap in res.results:
        acc += rmap["outp"]
    return np.ascontiguousarray(acc.T).reshape(B, S, D)


if __name__ == "__main__":
    rng = np.random.default_rng(0)
    scale = 1.0 / np.sqrt(D)
    x = rng.standard_normal((B, S, D), dtype=np.float32)
    Wq = rng.standard_normal((H, E, D), dtype=np.float32) * scale
    Wk = rng.standard_normal((H, E, D), dtype=np.float32) * scale
    Wv = rng.standard_normal((H, E, D), dtype=np.float32) * scale
    Wo = rng.standard_normal((D, D), dtype=np.float32) * scale
    out = kernel(x, Wq, Wk, Wv, Wo)
    print(out.shape, out.dtype, float(np.abs(out).max()))

